# revision 22
# baseline (speedup 1.0000x reference)
"""Trainium2 Bass kernel for nn_Attention_layer_67877663146058.

Computes attn = softmax((x @ Wq.T) @ (x @ Wk.T)^T * hd**-0.5)
for x [8, 1024, 768], W_qkv [2304, 768] -> out [8, 12, 1024, 1024] fp32.
The V third of W_qkv never reaches the output and is not loaded.

Sharding: batch-parallel across the 8 NeuronCores (core b handles batch b,
all 12 heads).

Design notes (evolved from trace analysis of two prior versions):
- v1 was HBM-bound (107% avg HBM util) writing the 50MB fp32 output per
  core. All data is now fp16; the output is written as fp16 scaled by
  1024 (softmax entries down to ~2e-6 would be fp16 subnormals; the
  x1024 shift keeps them normal) and the host upcast multiplies by the
  exact power-of-two 1/1024. HBM traffic: 58MB -> 29MB per core.
- ACT exp is then the pacer. All DVE reduce-variant instructions
  (tensor_scalar+accum, tensor_reduce, tensor_tensor_reduce, bn_stats)
  run at 1x (~1.13us per [128,1024] tile) - only plain copy/scalar ops
  reach the 4x 2-byte mode (~0.49us). GpSimd cannot reduce along the
  free dim at all. So row sums are split:
    * 3 qb-slices per f-tile ("A"): exp in tile-aligned [128,1024] spans
      with the free ACT accumulator (costs +187ns READ_ACCUMULATOR and
      span-overhead fragmentation on ACT),
    * 5 qb-slices per f-tile ("G"): exp in [128,1536] spans (amortizes
      the ~250-cycle ACT per-instruction overhead), sums via the 1x DVE
      identity tensor_scalar(mult,add,accum_out).
- The normalize multiply (y * (1024/sum), fused two-op tensor_scalar,
  4x on DVE at ~0.49us) is split DVE(5)/GpSimd(11) per f-tile; GpSimd's
  SW implementation takes ~1.32us but the engine is otherwise idle.
- Input DMAs are consolidated into 3 (x, W f-tile 0, W rest): each
  dma_start costs ~0.6us of serial descriptor-gen on the issuing
  sequencer, and 18 separate input DMAs delayed the first projection by
  ~9us in the v2 trace.
- PSUM: 2x [128,512] proj slots + 2x [128,1536] score slots = 8 banks.
- A dependency-free exp at t=0 preloads the ACT spline table (~2.7us).
  The projection matmul stream itself warms the PE HAM clock gate.
"""

import numpy as np
from contextlib import ExitStack

import concourse.bacc as bacc
import concourse.mybir as mybir
import concourse.tile as tile

# bass_utils imports antenv.axon_hooks when BASS_TRACE is set in the
# environment; some images ship an antenv stub without that module. Register
# a no-op fallback so tracing degrades gracefully instead of crashing.
try:
    from antenv.axon_hooks import get_axon_ntff_profile_hook as _g  # noqa: F401
except Exception:
    import sys as _sys
    import types as _types

    _m = _types.ModuleType("antenv.axon_hooks")
    _state = {"h": None}
    _m.set_axon_ntff_profile_hook = lambda h: _state.__setitem__("h", h)
    _m.get_axon_ntff_profile_hook = lambda: _state["h"]
    _sys.modules["antenv.axon_hooks"] = _m
    try:
        import antenv as _antenv

        _antenv.axon_hooks = _m
    except Exception:
        pass

from concourse.bass_utils import run_bass_kernel_spmd

B = 8          # batches == cores
N = 1024       # tokens
E = 768        # embed dim
H = 12         # heads
HD = 64        # head dim
FT = 6         # f-tiles (2 heads per f-tile)
ET = E // 128  # 6 e-tiles
SPAN = 1536    # G-region exp span (one PSUM score slot, 3 banks)
SCALE = HD ** -0.5
OUT_SCALE = 1024.0   # fp16 output holds out*1024 to avoid subnormals

# qb slices per f-tile whose sums come from the ACT accumulator
A_QBS = (0, 3, 6)
# tile indices (2*slot+tt over the fi's emission order) whose normalize
# multiply runs on DVE; the rest run on GpSimd
DVE_MULT = frozenset({5, 11})

_cache = {}


def _build():
    f32 = mybir.dt.float32
    f16 = mybir.dt.float16
    mult = mybir.AluOpType.mult
    add = mybir.AluOpType.add
    Exp = mybir.ActivationFunctionType.Exp
    nc = bacc.Bacc("TRN2", debug=False, num_devices=B)

    # inputs are packed partition-major on the host so each DMA moves
    # multi-KB contiguous runs per partition (128 fat descriptors/DMA)
    xP_d = nc.dram_tensor("xP", [128, ET * N], f16, kind="ExternalInput")
    wA_d = nc.dram_tensor("wA", [128, ET * 256], f16, kind="ExternalInput")
    wB_d = nc.dram_tensor("wB", [128, ET * 1280], f16, kind="ExternalInput")
    out_d = nc.dram_tensor("out", [FT * 8, 128, 2048], f16, kind="ExternalOutput")

    xP_src = xP_d.ap().rearrange("p (t n) -> p t n", t=ET)      # [128,6,1024]
    wA_src = wA_d.ap().rearrange("p (t c) -> p t c", t=ET)      # [128,6,256]
    wB_src = wB_d.ap().rearrange("p (t c) -> p t c", t=ET)      # [128,6,1280]
    out_ap = out_d.ap()

    with ExitStack() as ctx:
        tc = ctx.enter_context(tile.TileContext(nc))
        statics = ctx.enter_context(tc.tile_pool(name="statics", bufs=1))
        ypool = ctx.enter_context(tc.tile_pool(name="ypool", bufs=3))
        spool = ctx.enter_context(tc.tile_pool(name="spool", bufs=2))
        psum = ctx.enter_context(tc.tile_pool(name="psum", bufs=2, space="PSUM"))

        xt = statics.tile([128, ET, N], f16, tag="xt", name="xt")
        # W columns split into two contiguous tiles (f-tile 0 / rest) so
        # each input DMA moves multi-KB runs per partition
        wt0 = statics.tile([128, ET, 256], f16, tag="wt0", name="wt0")
        wtR = statics.tile([128, ET, 1280], f16, tag="wtR", name="wtR")
        # qkt[:, fi, 0, :] = K^T of f-tile fi, qkt[:, fi, 1, :] = Q^T
        qkt = statics.tile([128, FT, 2, N], f16, tag="qkt", name="qkt")

        def w_block(fi, kq, ei):
            if fi == 0:
                return wt0[:, ei, kq * 128:(kq + 1) * 128]
            c = (fi - 1) * 256 + kq * 128
            return wtR[:, ei, c:c + 128]

        # ACT table preload: dependency-free exp at t=0 pulls the ~2.7us
        # ACT_TABLE_LOAD off the critical path of the first real exp.
        warm = spool.tile([128, 1], f32, tag="warm", name="warm")
        nc.vector.memset(warm, 0.0)
        nc.scalar.activation(warm, warm, Exp)

        # PE HAM warm-up: ~10 dummy matmuls (~4.3us at the cold clock)
        # flip the PE clock gate to 2.4 GHz while the input DMAs run.
        wl = statics.tile([128, 128], f16, tag="wl", name="wl")
        wr = statics.tile([128, 512], f16, tag="wr", name="wr")
        nc.vector.memset(wl, 0.0)
        nc.vector.memset(wr, 0.0)
        pw = psum.tile([128, 512], f32, tag="pp", name="pw")
        for _ in range(10):
            nc.tensor.matmul(pw, lhsT=wl, rhs=wr, start=True, stop=True,
                             skip_group_check=True)

        # Consolidated input loads (4 fat DMAs, 128 descriptors each): x
        # halves and f-tile-0 W columns first so the first projection can
        # start as early as possible.
        nc.sync.dma_start(xt[:, 0:3, :], xP_src[:, 0:3, :])
        nc.sync.dma_start(wt0, wA_src)
        nc.sync.dma_start(xt[:, 3:6, :], xP_src[:, 3:6, :])
        nc.sync.dma_start(wtR, wB_src)

        QUARTERS = [(0, 0), (0, 1), (1, 0), (1, 1)]  # K halves first

        def proj_steps(fi):
            # The projection for f-tile fi as a list of single-instruction
            # closures (6 accumulating matmuls + 1 evacuation cast per
            # quarter). Threading these one or two at a time between score
            # spans keeps the PE from blocking the ACT-paced span stream
            # for ~2us at a stretch (accumulation groups don't need to be
            # contiguous in the PE program; they only own their PSUM bank).
            steps = []
            for kq, half in QUARTERS:
                holder = []
                for ei in range(ET):
                    def mm_step(kq=kq, half=half, ei=ei, holder=holder):
                        if ei == 0:
                            holder.append(psum.tile(
                                [128, 512], f32, tag="pp",
                                name=f"pp{fi}_{kq}_{half}",
                            ))
                        nc.tensor.matmul(
                            holder[0],
                            lhsT=w_block(fi, kq, ei),
                            rhs=xt[:, ei, half * 512:(half + 1) * 512],
                            start=(ei == 0),
                            stop=(ei == ET - 1),
                            skip_group_check=True,
                        )
                    steps.append(mm_step)

                def cast_step(kq=kq, half=half, holder=holder):
                    nc.vector.tensor_copy(
                        qkt[:, fi, kq, half * 512:(half + 1) * 512], holder[0]
                    )
                steps.append(cast_step)
            return steps

        def score_mm(ps, off, fi, qb, hh, nh):
            lo = 64 * hh
            nc.tensor.matmul(
                ps[:, off:off + 512],
                lhsT=qkt[lo:lo + 64, fi, 1, qb * 128:(qb + 1) * 128],
                rhs=qkt[lo:lo + 64, fi, 0, nh * 512:(nh + 1) * 512],
                start=True,
                stop=True,
                tile_position=(lo, 0),
            )

        def emit_attn(fi, interleave):
            # y slot s (0..7) holds logical qb slot_qb[s]. A-slices (ACT
            # accumulator) lead so the G region is span-contiguous - except
            # in the last f-tile, where A-slices trail so the kernel tail
            # after the final exp is just recip+mult+DMA of one slice.
            a_last = fi == FT - 1
            g_qbs = [q for q in range(8) if q not in A_QBS]
            slot_qb = (g_qbs + list(A_QBS)) if a_last else (list(A_QBS) + g_qbs)
            a_slots = range(5, 8) if a_last else range(0, 3)
            g0 = 0 if a_last else len(A_QBS) * 2048
            y = ypool.tile([128, 16 * N], f16, tag="y", name=f"y{fi}")
            sums = spool.tile([128, 16], f32, tag="sums", name=f"sm{fi}")
            rec = spool.tile([128, 16], f32, tag="rec", name=f"rc{fi}")
            pending = list(interleave)

            def pull(k):
                for fn in pending[:k]:
                    fn()
                del pending[:k]

            def finish_slice(s):
                pull(1)
                qb = slot_qb[s]
                t0 = 2 * s
                nc.vector.reciprocal(rec[:, t0:t0 + 2], sums[:, t0:t0 + 2])
                for tt in (t0, t0 + 1):
                    yt = y[:, tt * N:(tt + 1) * N]
                    dve = (s >= 5) if a_last else (tt in DVE_MULT)
                    eng = nc.vector if dve else nc.gpsimd
                    eng.tensor_scalar(yt, yt, rec[:, tt:tt + 1], OUT_SCALE, mult, mult)
                nc.sync.dma_start(
                    out_ap[fi * 8 + qb], y[:, s * 2048:(s + 1) * 2048]
                )

            def emit_a_region():
                for s in a_slots:
                    qb = slot_qb[s]
                    for tt in (2 * s, 2 * s + 1):
                        hh = tt % 2
                        ps = psum.tile([128, SPAN], f32, tag="ps", name=f"psA{fi}_{tt}")
                        for nh in range(2):
                            score_mm(ps, nh * 512, fi, qb, hh, nh)
                        nc.scalar.activation(
                            y[:, tt * N:(tt + 1) * N], ps[:, 0:N], Exp, scale=SCALE,
                            accum_out=sums[:, tt:tt + 1],
                        )
                        pull(2)
                    finish_slice(s)

            def emit_g_region():
                done = g0
                for c0 in range(g0, g0 + len(g_qbs) * 2048, SPAN):
                    L = min(SPAN, g0 + len(g_qbs) * 2048 - c0)
                    ps = psum.tile([128, SPAN], f32, tag="ps", name=f"psG{fi}_{c0}")
                    for off in range(0, L, 512):
                        g = c0 + off
                        s = g // 2048
                        score_mm(ps, off, fi, slot_qb[s], (g // 1024) % 2, (g // 512) % 2)
                    nc.scalar.activation(
                        y[:, c0:c0 + L], ps[:, 0:L], Exp, scale=SCALE,
                    )
                    pull(2)
                    new_done = ((c0 + L) // N) * N
                    for tt in range(done // N, new_done // N):
                        yt = y[:, tt * N:(tt + 1) * N]
                        nc.vector.tensor_scalar(
                            yt, yt, 1.0, 0.0, mult, add, accum_out=sums[:, tt:tt + 1],
                        )
                        if tt % 2 == 1:
                            finish_slice(tt // 2)
                    done = new_done

            if a_last:
                emit_g_region()
                emit_a_region()
            else:
                emit_a_region()
                emit_g_region()

        # fi0 projection upfront; fi+1's projection threads into fi's attn
        # stream one instruction at a time.
        for fn in proj_steps(0):
            fn()
        for fi in range(FT):
            interleave = proj_steps(fi + 1) if fi + 1 < FT else []
            emit_attn(fi, interleave)

    nc.compile()
    return nc


def _prep_inputs(x, W_qkv):
    x = np.asarray(x, dtype=np.float32)
    W = np.asarray(W_qkv, dtype=np.float32)
    # per-fi W column blocks [K_fi (128) | Q_fi (128)], then packed
    # partition-major: w[p, ei, c] = wT[ei*128+p, c]
    wq = W[0:768].reshape(FT, 128, E)        # Q blocks per f-tile
    wk = W[768:1536].reshape(FT, 128, E)     # K blocks per f-tile
    wkq = np.stack([wk, wq], axis=1)         # [fi, kq, 128, e]
    wT = wkq.transpose(3, 0, 1, 2).reshape(E, 2 * 128 * FT)  # [e, cols]
    wP = wT.reshape(ET, 128, 2 * 128 * FT).transpose(1, 0, 2)  # [p, ei, cols]
    wA = np.ascontiguousarray(wP[:, :, 0:256].reshape(128, -1)).astype(np.float16)
    wB = np.ascontiguousarray(wP[:, :, 256:1536].reshape(128, -1)).astype(np.float16)
    in_maps = []
    for b in range(B):
        xT = x[b].T                           # [e, n]
        xP = np.ascontiguousarray(
            xT.reshape(ET, 128, N).transpose(1, 0, 2).reshape(128, -1)
        ).astype(np.float16)
        in_maps.append({"xP": xP, "wA": wA, "wB": wB})
    return in_maps


def _postprocess(res):
    outs = []
    inv = np.float32(1.0 / OUT_SCALE)
    for r in res.results:
        buf = r["out"]            # [48, 128, 2048] fp16, = out*1024
        buf = np.asarray(buf)
        if buf.dtype != np.float16:
            buf = buf.view(np.float16)
        full = buf.reshape(FT, 8, 128, 2, N).transpose(0, 3, 1, 2, 4)
        full = full.reshape(H, N, N).astype(np.float32) * inv
        outs.append(full)
    return np.stack(outs, axis=0)


def _run(x, W_qkv, trace=False):
    if "nc" not in _cache:
        _cache["nc"] = _build()
    nc = _cache["nc"]
    in_maps = _prep_inputs(x, W_qkv)
    res = run_bass_kernel_spmd(nc, in_maps, core_ids=list(range(B)), trace=trace)
    return _postprocess(res), res


def kernel(x, W_qkv):
    return _run(x, W_qkv)[0]


# revision 23
# speedup vs baseline: 1.0177x; 1.0177x over previous
"""Trainium2 Bass kernel for nn_Attention_layer_67877663146058.

Computes attn = softmax((x @ Wq.T) @ (x @ Wk.T)^T * hd**-0.5)
for x [8, 1024, 768], W_qkv [2304, 768] -> out [8, 12, 1024, 1024] fp32.
The V third of W_qkv never reaches the output and is not loaded.

Sharding: batch-parallel across the 8 NeuronCores (core b handles batch b,
all 12 heads).

Design notes (evolved from trace analysis of two prior versions):
- v1 was HBM-bound (107% avg HBM util) writing the 50MB fp32 output per
  core. All data is now fp16; the output is written as fp16 scaled by
  1024 (softmax entries down to ~2e-6 would be fp16 subnormals; the
  x1024 shift keeps them normal) and the host upcast multiplies by the
  exact power-of-two 1/1024. HBM traffic: 58MB -> 29MB per core.
- ACT exp is then the pacer. All DVE reduce-variant instructions
  (tensor_scalar+accum, tensor_reduce, tensor_tensor_reduce, bn_stats)
  run at 1x (~1.13us per [128,1024] tile) - only plain copy/scalar ops
  reach the 4x 2-byte mode (~0.49us). GpSimd cannot reduce along the
  free dim at all. So row sums are split:
    * 3 qb-slices per f-tile ("A"): exp in tile-aligned [128,1024] spans
      with the free ACT accumulator (costs +187ns READ_ACCUMULATOR and
      span-overhead fragmentation on ACT),
    * 5 qb-slices per f-tile ("G"): exp in [128,1536] spans (amortizes
      the ~250-cycle ACT per-instruction overhead), sums via the 1x DVE
      identity tensor_scalar(mult,add,accum_out).
- The normalize multiply (y * (1024/sum), fused two-op tensor_scalar,
  4x on DVE at ~0.49us) is split DVE(5)/GpSimd(11) per f-tile; GpSimd's
  SW implementation takes ~1.32us but the engine is otherwise idle.
- Input DMAs are consolidated into 3 (x, W f-tile 0, W rest): each
  dma_start costs ~0.6us of serial descriptor-gen on the issuing
  sequencer, and 18 separate input DMAs delayed the first projection by
  ~9us in the v2 trace.
- PSUM: 2x [128,512] proj slots + 2x [128,1536] score slots = 8 banks.
- A dependency-free exp at t=0 preloads the ACT spline table (~2.7us).
  The projection matmul stream itself warms the PE HAM clock gate.
"""

import numpy as np
from contextlib import ExitStack

import concourse.bacc as bacc
import concourse.mybir as mybir
import concourse.tile as tile

# bass_utils imports antenv.axon_hooks when BASS_TRACE is set in the
# environment; some images ship an antenv stub without that module. Register
# a no-op fallback so tracing degrades gracefully instead of crashing.
try:
    from antenv.axon_hooks import get_axon_ntff_profile_hook as _g  # noqa: F401
except Exception:
    import sys as _sys
    import types as _types

    _m = _types.ModuleType("antenv.axon_hooks")
    _state = {"h": None}
    _m.set_axon_ntff_profile_hook = lambda h: _state.__setitem__("h", h)
    _m.get_axon_ntff_profile_hook = lambda: _state["h"]
    _sys.modules["antenv.axon_hooks"] = _m
    try:
        import antenv as _antenv

        _antenv.axon_hooks = _m
    except Exception:
        pass

from concourse.bass_utils import run_bass_kernel_spmd

B = 8          # batches == cores
N = 1024       # tokens
E = 768        # embed dim
H = 12         # heads
HD = 64        # head dim
FT = 6         # f-tiles (2 heads per f-tile)
ET = E // 128  # 6 e-tiles
SPAN = 1536    # G-region exp span (one PSUM score slot, 3 banks)
SCALE = HD ** -0.5
OUT_SCALE = 1024.0   # fp16 output holds out*1024 to avoid subnormals

# qb slices per f-tile whose sums come from the ACT accumulator
A_QBS = (0, 3, 6)
# tile indices (2*slot+tt over the fi's emission order) whose normalize
# multiply runs on DVE; the rest run on GpSimd
DVE_MULT = frozenset({5, 11})

_cache = {}


def _build():
    f32 = mybir.dt.float32
    f16 = mybir.dt.float16
    mult = mybir.AluOpType.mult
    add = mybir.AluOpType.add
    Exp = mybir.ActivationFunctionType.Exp
    nc = bacc.Bacc("TRN2", debug=False, num_devices=B)

    # inputs are packed partition-major on the host so each DMA moves
    # multi-KB contiguous runs per partition (128 fat descriptors/DMA)
    xP_d = nc.dram_tensor("xP", [128, ET * N], f16, kind="ExternalInput")
    wA_d = nc.dram_tensor("wA", [128, ET * 256], f16, kind="ExternalInput")
    wB_d = nc.dram_tensor("wB", [128, ET * 1280], f16, kind="ExternalInput")
    out_d = nc.dram_tensor("out", [FT * 8, 128, 2048], f16, kind="ExternalOutput")

    xP_src = xP_d.ap().rearrange("p (t n) -> p t n", t=ET)      # [128,6,1024]
    wA_src = wA_d.ap().rearrange("p (t c) -> p t c", t=ET)      # [128,6,256]
    wB_src = wB_d.ap().rearrange("p (t c) -> p t c", t=ET)      # [128,6,1280]
    out_ap = out_d.ap()

    with ExitStack() as ctx:
        tc = ctx.enter_context(tile.TileContext(nc))
        statics = ctx.enter_context(tc.tile_pool(name="statics", bufs=1))
        ypool = ctx.enter_context(tc.tile_pool(name="ypool", bufs=2))
        spool = ctx.enter_context(tc.tile_pool(name="spool", bufs=2))
        psum = ctx.enter_context(tc.tile_pool(name="psum", bufs=2, space="PSUM"))

        xt = statics.tile([128, ET, N], f16, tag="xt", name="xt")
        # W columns split into two contiguous tiles (f-tile 0 / rest) so
        # each input DMA moves multi-KB runs per partition
        wt0 = statics.tile([128, ET, 256], f16, tag="wt0", name="wt0")
        wtR = statics.tile([128, ET, 1280], f16, tag="wtR", name="wtR")
        # qkt[:, fi, 0, :] = K^T of f-tile fi, qkt[:, fi, 1, :] = Q^T
        qkt = statics.tile([128, FT, 2, N], f16, tag="qkt", name="qkt")

        def w_block(fi, kq, ei):
            if fi == 0:
                return wt0[:, ei, kq * 128:(kq + 1) * 128]
            c = (fi - 1) * 256 + kq * 128
            return wtR[:, ei, c:c + 128]

        # ACT table preload: dependency-free exp at t=0 pulls the ~2.7us
        # ACT_TABLE_LOAD off the critical path of the first real exp.
        warm = spool.tile([128, 1], f32, tag="warm", name="warm")
        nc.vector.memset(warm, 0.0)
        nc.scalar.activation(warm, warm, Exp)

        # Consolidated input loads (4 fat DMAs, 128 descriptors each): x
        # halves and f-tile-0 W columns first so the first projection can
        # start as early as possible.
        nc.sync.dma_start(xt[:, 0:3, :], xP_src[:, 0:3, :])
        nc.sync.dma_start(wt0, wA_src)
        nc.sync.dma_start(xt[:, 3:6, :], xP_src[:, 3:6, :])
        nc.sync.dma_start(wtR, wB_src)

        QUARTERS = [(0, 0), (0, 1), (1, 0), (1, 1)]  # K halves first

        def proj_steps(fi):
            # The projection for f-tile fi as a list of single-instruction
            # closures (6 accumulating matmuls + 1 evacuation cast per
            # quarter). Threading these one or two at a time between score
            # spans keeps the PE from blocking the ACT-paced span stream
            # for ~2us at a stretch (accumulation groups don't need to be
            # contiguous in the PE program; they only own their PSUM bank).
            steps = []
            for kq, half in QUARTERS:
                holder = []
                for ei in range(ET):
                    def mm_step(kq=kq, half=half, ei=ei, holder=holder):
                        if ei == 0:
                            holder.append(psum.tile(
                                [128, 512], f32, tag="pp",
                                name=f"pp{fi}_{kq}_{half}",
                            ))
                        nc.tensor.matmul(
                            holder[0],
                            lhsT=w_block(fi, kq, ei),
                            rhs=xt[:, ei, half * 512:(half + 1) * 512],
                            start=(ei == 0),
                            stop=(ei == ET - 1),
                            skip_group_check=True,
                        )
                    steps.append(mm_step)

                def cast_step(kq=kq, half=half, holder=holder):
                    nc.vector.tensor_copy(
                        qkt[:, fi, kq, half * 512:(half + 1) * 512], holder[0]
                    )
                steps.append(cast_step)
            return steps

        def score_mm(ps, off, fi, qb, hh, nh):
            lo = 64 * hh
            nc.tensor.matmul(
                ps[:, off:off + 512],
                lhsT=qkt[lo:lo + 64, fi, 1, qb * 128:(qb + 1) * 128],
                rhs=qkt[lo:lo + 64, fi, 0, nh * 512:(nh + 1) * 512],
                start=True,
                stop=True,
                tile_position=(lo, 0),
            )

        def emit_attn(fi, interleave):
            # y slot s (0..7) holds logical qb slot_qb[s]. A-slices (ACT
            # accumulator) lead so the G region is span-contiguous - except
            # in the last f-tile, where A-slices trail so the kernel tail
            # after the final exp is just recip+mult+DMA of one slice.
            a_last = fi == FT - 1
            g_qbs = [q for q in range(8) if q not in A_QBS]
            slot_qb = (g_qbs + list(A_QBS)) if a_last else (list(A_QBS) + g_qbs)
            a_slots = range(5, 8) if a_last else range(0, 3)
            g0 = 0 if a_last else len(A_QBS) * 2048
            y = ypool.tile([128, 16 * N], f16, tag="y", name=f"y{fi}")
            sums = spool.tile([128, 16], f32, tag="sums", name=f"sm{fi}")
            rec = spool.tile([128, 16], f32, tag="rec", name=f"rc{fi}")
            pending = list(interleave)

            def pull(k):
                for fn in pending[:k]:
                    fn()
                del pending[:k]

            def finish_slice(s):
                pull(1)
                qb = slot_qb[s]
                t0 = 2 * s
                nc.vector.reciprocal(rec[:, t0:t0 + 2], sums[:, t0:t0 + 2])
                for tt in (t0, t0 + 1):
                    yt = y[:, tt * N:(tt + 1) * N]
                    dve = (s >= 5) if a_last else (tt in DVE_MULT)
                    eng = nc.vector if dve else nc.gpsimd
                    eng.tensor_scalar(yt, yt, rec[:, tt:tt + 1], OUT_SCALE, mult, mult)
                nc.sync.dma_start(
                    out_ap[fi * 8 + qb], y[:, s * 2048:(s + 1) * 2048]
                )

            def emit_a_region():
                for s in a_slots:
                    qb = slot_qb[s]
                    for tt in (2 * s, 2 * s + 1):
                        hh = tt % 2
                        ps = psum.tile([128, SPAN], f32, tag="ps", name=f"psA{fi}_{tt}")
                        for nh in range(2):
                            score_mm(ps, nh * 512, fi, qb, hh, nh)
                        nc.scalar.activation(
                            y[:, tt * N:(tt + 1) * N], ps[:, 0:N], Exp, scale=SCALE,
                            accum_out=sums[:, tt:tt + 1],
                        )
                        pull(2)
                    finish_slice(s)

            def emit_g_region():
                done = g0
                for c0 in range(g0, g0 + len(g_qbs) * 2048, SPAN):
                    L = min(SPAN, g0 + len(g_qbs) * 2048 - c0)
                    ps = psum.tile([128, SPAN], f32, tag="ps", name=f"psG{fi}_{c0}")
                    for off in range(0, L, 512):
                        g = c0 + off
                        s = g // 2048
                        score_mm(ps, off, fi, slot_qb[s], (g // 1024) % 2, (g // 512) % 2)
                    nc.scalar.activation(
                        y[:, c0:c0 + L], ps[:, 0:L], Exp, scale=SCALE,
                    )
                    pull(2)
                    new_done = ((c0 + L) // N) * N
                    for tt in range(done // N, new_done // N):
                        yt = y[:, tt * N:(tt + 1) * N]
                        nc.vector.tensor_scalar(
                            yt, yt, 1.0, 0.0, mult, add, accum_out=sums[:, tt:tt + 1],
                        )
                        if tt % 2 == 1:
                            finish_slice(tt // 2)
                    done = new_done

            if a_last:
                emit_g_region()
                emit_a_region()
            else:
                emit_a_region()
                emit_g_region()

        # fi0 projection upfront; fi+1's projection threads into fi's attn
        # stream one instruction at a time.
        for fn in proj_steps(0):
            fn()
        for fi in range(FT):
            interleave = proj_steps(fi + 1) if fi + 1 < FT else []
            emit_attn(fi, interleave)

    nc.compile()
    return nc


def _prep_inputs(x, W_qkv):
    x = np.asarray(x, dtype=np.float32)
    W = np.asarray(W_qkv, dtype=np.float32)
    # per-fi W column blocks [K_fi (128) | Q_fi (128)], then packed
    # partition-major: w[p, ei, c] = wT[ei*128+p, c]
    wq = W[0:768].reshape(FT, 128, E)        # Q blocks per f-tile
    wk = W[768:1536].reshape(FT, 128, E)     # K blocks per f-tile
    wkq = np.stack([wk, wq], axis=1)         # [fi, kq, 128, e]
    wT = wkq.transpose(3, 0, 1, 2).reshape(E, 2 * 128 * FT)  # [e, cols]
    wP = wT.reshape(ET, 128, 2 * 128 * FT).transpose(1, 0, 2)  # [p, ei, cols]
    wA = np.ascontiguousarray(wP[:, :, 0:256].reshape(128, -1)).astype(np.float16)
    wB = np.ascontiguousarray(wP[:, :, 256:1536].reshape(128, -1)).astype(np.float16)
    in_maps = []
    for b in range(B):
        xT = x[b].T                           # [e, n]
        xP = np.ascontiguousarray(
            xT.reshape(ET, 128, N).transpose(1, 0, 2).reshape(128, -1)
        ).astype(np.float16)
        in_maps.append({"xP": xP, "wA": wA, "wB": wB})
    return in_maps


def _postprocess(res):
    outs = []
    inv = np.float32(1.0 / OUT_SCALE)
    for r in res.results:
        buf = r["out"]            # [48, 128, 2048] fp16, = out*1024
        buf = np.asarray(buf)
        if buf.dtype != np.float16:
            buf = buf.view(np.float16)
        full = buf.reshape(FT, 8, 128, 2, N).transpose(0, 3, 1, 2, 4)
        full = full.reshape(H, N, N).astype(np.float32) * inv
        outs.append(full)
    return np.stack(outs, axis=0)


def _run(x, W_qkv, trace=False):
    if "nc" not in _cache:
        _cache["nc"] = _build()
    nc = _cache["nc"]
    in_maps = _prep_inputs(x, W_qkv)
    res = run_bass_kernel_spmd(nc, in_maps, core_ids=list(range(B)), trace=trace)
    return _postprocess(res), res


def kernel(x, W_qkv):
    return _run(x, W_qkv)[0]


# revision 24
# speedup vs baseline: 1.0259x; 1.0081x over previous
"""Trainium2 Bass kernel for nn_Attention_layer_67877663146058.

Computes attn = softmax((x @ Wq.T) @ (x @ Wk.T)^T * hd**-0.5)
for x [8, 1024, 768], W_qkv [2304, 768] -> out [8, 12, 1024, 1024] fp32.
The V third of W_qkv never reaches the output and is not loaded.

Sharding: batch-parallel across the 8 NeuronCores (core b handles batch b,
all 12 heads).

Design notes (evolved from trace analysis of two prior versions):
- v1 was HBM-bound (107% avg HBM util) writing the 50MB fp32 output per
  core. All data is now fp16; the output is written as fp16 scaled by
  1024 (softmax entries down to ~2e-6 would be fp16 subnormals; the
  x1024 shift keeps them normal) and the host upcast multiplies by the
  exact power-of-two 1/1024. HBM traffic: 58MB -> 29MB per core.
- ACT exp is then the pacer. All DVE reduce-variant instructions
  (tensor_scalar+accum, tensor_reduce, tensor_tensor_reduce, bn_stats)
  run at 1x (~1.13us per [128,1024] tile) - only plain copy/scalar ops
  reach the 4x 2-byte mode (~0.49us). GpSimd cannot reduce along the
  free dim at all. So row sums are split:
    * 3 qb-slices per f-tile ("A"): exp in tile-aligned [128,1024] spans
      with the free ACT accumulator (costs +187ns READ_ACCUMULATOR and
      span-overhead fragmentation on ACT),
    * 5 qb-slices per f-tile ("G"): exp in [128,1536] spans (amortizes
      the ~250-cycle ACT per-instruction overhead), sums via the 1x DVE
      identity tensor_scalar(mult,add,accum_out).
- The normalize multiply (y * (1024/sum), fused two-op tensor_scalar,
  4x on DVE at ~0.49us) is split DVE(5)/GpSimd(11) per f-tile; GpSimd's
  SW implementation takes ~1.32us but the engine is otherwise idle.
- Input DMAs are consolidated into 3 (x, W f-tile 0, W rest): each
  dma_start costs ~0.6us of serial descriptor-gen on the issuing
  sequencer, and 18 separate input DMAs delayed the first projection by
  ~9us in the v2 trace.
- PSUM: 2x [128,512] proj slots + 2x [128,1536] score slots = 8 banks.
- A dependency-free exp at t=0 preloads the ACT spline table (~2.7us).
  The projection matmul stream itself warms the PE HAM clock gate.
"""

import numpy as np
from contextlib import ExitStack

import concourse.bacc as bacc
import concourse.mybir as mybir
import concourse.tile as tile

# bass_utils imports antenv.axon_hooks when BASS_TRACE is set in the
# environment; some images ship an antenv stub without that module. Register
# a no-op fallback so tracing degrades gracefully instead of crashing.
try:
    from antenv.axon_hooks import get_axon_ntff_profile_hook as _g  # noqa: F401
except Exception:
    import sys as _sys
    import types as _types

    _m = _types.ModuleType("antenv.axon_hooks")
    _state = {"h": None}
    _m.set_axon_ntff_profile_hook = lambda h: _state.__setitem__("h", h)
    _m.get_axon_ntff_profile_hook = lambda: _state["h"]
    _sys.modules["antenv.axon_hooks"] = _m
    try:
        import antenv as _antenv

        _antenv.axon_hooks = _m
    except Exception:
        pass

from concourse.bass_utils import run_bass_kernel_spmd

B = 8          # batches == cores
N = 1024       # tokens
E = 768        # embed dim
H = 12         # heads
HD = 64        # head dim
FT = 6         # f-tiles (2 heads per f-tile)
ET = E // 128  # 6 e-tiles
SPAN = 1536    # G-region exp span (one PSUM score slot, 3 banks)
SCALE = HD ** -0.5
OUT_SCALE = 1024.0   # fp16 output holds out*1024 to avoid subnormals

# qb slices per f-tile whose sums come from the ACT accumulator
A_QBS = (0, 3, 6)
# tile indices (2*slot+tt over the fi's emission order) whose normalize
# multiply runs on DVE; the rest run on GpSimd
DVE_MULT = frozenset({5})

_cache = {}


def _build():
    f32 = mybir.dt.float32
    f16 = mybir.dt.float16
    mult = mybir.AluOpType.mult
    add = mybir.AluOpType.add
    Exp = mybir.ActivationFunctionType.Exp
    nc = bacc.Bacc("TRN2", debug=False, num_devices=B)

    # inputs are packed partition-major on the host so each DMA moves
    # multi-KB contiguous runs per partition (128 fat descriptors/DMA)
    xP_d = nc.dram_tensor("xP", [128, ET * N], f16, kind="ExternalInput")
    wA_d = nc.dram_tensor("wA", [128, ET * 256], f16, kind="ExternalInput")
    wB_d = nc.dram_tensor("wB", [128, ET * 1280], f16, kind="ExternalInput")
    out_d = nc.dram_tensor("out", [FT * 8, 128, 2048], f16, kind="ExternalOutput")

    xP_src = xP_d.ap().rearrange("p (t n) -> p t n", t=ET)      # [128,6,1024]
    wA_src = wA_d.ap().rearrange("p (t c) -> p t c", t=ET)      # [128,6,256]
    wB_src = wB_d.ap().rearrange("p (t c) -> p t c", t=ET)      # [128,6,1280]
    out_ap = out_d.ap()

    with ExitStack() as ctx:
        tc = ctx.enter_context(tile.TileContext(nc))
        statics = ctx.enter_context(tc.tile_pool(name="statics", bufs=1))
        ypool = ctx.enter_context(tc.tile_pool(name="ypool", bufs=2))
        spool = ctx.enter_context(tc.tile_pool(name="spool", bufs=2))
        psum = ctx.enter_context(tc.tile_pool(name="psum", bufs=2, space="PSUM"))

        xt = statics.tile([128, ET, N], f16, tag="xt", name="xt")
        # W columns split into two contiguous tiles (f-tile 0 / rest) so
        # each input DMA moves multi-KB runs per partition
        wt0 = statics.tile([128, ET, 256], f16, tag="wt0", name="wt0")
        wtR = statics.tile([128, ET, 1280], f16, tag="wtR", name="wtR")
        # qkt[:, fi, 0, :] = K^T of f-tile fi, qkt[:, fi, 1, :] = Q^T
        qkt = statics.tile([128, FT, 2, N], f16, tag="qkt", name="qkt")

        def w_block(fi, kq, ei):
            if fi == 0:
                return wt0[:, ei, kq * 128:(kq + 1) * 128]
            c = (fi - 1) * 256 + kq * 128
            return wtR[:, ei, c:c + 128]

        # ACT table preload: dependency-free exp at t=0 pulls the ~2.7us
        # ACT_TABLE_LOAD off the critical path of the first real exp.
        warm = spool.tile([128, 1], f32, tag="warm", name="warm")
        nc.vector.memset(warm, 0.0)
        nc.scalar.activation(warm, warm, Exp)

        # Consolidated input loads (4 fat DMAs, 128 descriptors each): x
        # halves and f-tile-0 W columns first so the first projection can
        # start as early as possible.
        nc.sync.dma_start(xt[:, 0:3, :], xP_src[:, 0:3, :])
        nc.sync.dma_start(wt0, wA_src)
        nc.sync.dma_start(xt[:, 3:6, :], xP_src[:, 3:6, :])
        nc.sync.dma_start(wtR, wB_src)

        QUARTERS = [(0, 0), (0, 1), (1, 0), (1, 1)]  # K halves first

        def proj_steps(fi):
            # The projection for f-tile fi as a list of single-instruction
            # closures (6 accumulating matmuls + 1 evacuation cast per
            # quarter). Threading these one or two at a time between score
            # spans keeps the PE from blocking the ACT-paced span stream
            # for ~2us at a stretch (accumulation groups don't need to be
            # contiguous in the PE program; they only own their PSUM bank).
            steps = []
            for kq, half in QUARTERS:
                holder = []
                for ei in range(ET):
                    def mm_step(kq=kq, half=half, ei=ei, holder=holder):
                        if ei == 0:
                            holder.append(psum.tile(
                                [128, 512], f32, tag="pp",
                                name=f"pp{fi}_{kq}_{half}",
                            ))
                        nc.tensor.matmul(
                            holder[0],
                            lhsT=w_block(fi, kq, ei),
                            rhs=xt[:, ei, half * 512:(half + 1) * 512],
                            start=(ei == 0),
                            stop=(ei == ET - 1),
                            skip_group_check=True,
                        )
                    steps.append(mm_step)

                def cast_step(kq=kq, half=half, holder=holder):
                    nc.vector.tensor_copy(
                        qkt[:, fi, kq, half * 512:(half + 1) * 512], holder[0]
                    )
                steps.append(cast_step)
            return steps

        def score_mm(ps, off, fi, qb, hh, nh):
            lo = 64 * hh
            nc.tensor.matmul(
                ps[:, off:off + 512],
                lhsT=qkt[lo:lo + 64, fi, 1, qb * 128:(qb + 1) * 128],
                rhs=qkt[lo:lo + 64, fi, 0, nh * 512:(nh + 1) * 512],
                start=True,
                stop=True,
                tile_position=(lo, 0),
            )

        def emit_attn(fi, interleave):
            # y slot s (0..7) holds logical qb slot_qb[s]. A-slices (ACT
            # accumulator) lead so the G region is span-contiguous - except
            # in the last f-tile, where A-slices trail so the kernel tail
            # after the final exp is just recip+mult+DMA of one slice.
            a_last = fi == FT - 1
            g_qbs = [q for q in range(8) if q not in A_QBS]
            slot_qb = (g_qbs + list(A_QBS)) if a_last else (list(A_QBS) + g_qbs)
            a_slots = range(5, 8) if a_last else range(0, 3)
            g0 = 0 if a_last else len(A_QBS) * 2048
            y = ypool.tile([128, 16 * N], f16, tag="y", name=f"y{fi}")
            sums = spool.tile([128, 16], f32, tag="sums", name=f"sm{fi}")
            rec = spool.tile([128, 16], f32, tag="rec", name=f"rc{fi}")
            pending = list(interleave)

            def pull(k):
                for fn in pending[:k]:
                    fn()
                del pending[:k]

            def finish_slice(s):
                pull(1)
                qb = slot_qb[s]
                t0 = 2 * s
                nc.vector.reciprocal(rec[:, t0:t0 + 2], sums[:, t0:t0 + 2])
                for tt in (t0, t0 + 1):
                    yt = y[:, tt * N:(tt + 1) * N]
                    dve = (s >= 5) if a_last else (tt in DVE_MULT)
                    eng = nc.vector if dve else nc.gpsimd
                    eng.tensor_scalar(yt, yt, rec[:, tt:tt + 1], OUT_SCALE, mult, mult)
                nc.sync.dma_start(
                    out_ap[fi * 8 + qb], y[:, s * 2048:(s + 1) * 2048]
                )

            def emit_a_region():
                for s in a_slots:
                    qb = slot_qb[s]
                    for tt in (2 * s, 2 * s + 1):
                        hh = tt % 2
                        ps = psum.tile([128, SPAN], f32, tag="ps", name=f"psA{fi}_{tt}")
                        for nh in range(2):
                            score_mm(ps, nh * 512, fi, qb, hh, nh)
                        nc.scalar.activation(
                            y[:, tt * N:(tt + 1) * N], ps[:, 0:N], Exp, scale=SCALE,
                            accum_out=sums[:, tt:tt + 1],
                        )
                        pull(2)
                    finish_slice(s)

            def emit_g_region():
                done = g0
                for c0 in range(g0, g0 + len(g_qbs) * 2048, SPAN):
                    L = min(SPAN, g0 + len(g_qbs) * 2048 - c0)
                    ps = psum.tile([128, SPAN], f32, tag="ps", name=f"psG{fi}_{c0}")
                    for off in range(0, L, 512):
                        g = c0 + off
                        s = g // 2048
                        score_mm(ps, off, fi, slot_qb[s], (g // 1024) % 2, (g // 512) % 2)
                    nc.scalar.activation(
                        y[:, c0:c0 + L], ps[:, 0:L], Exp, scale=SCALE,
                    )
                    pull(2)
                    new_done = ((c0 + L) // N) * N
                    for tt in range(done // N, new_done // N):
                        yt = y[:, tt * N:(tt + 1) * N]
                        nc.vector.tensor_scalar(
                            yt, yt, 1.0, 0.0, mult, add, accum_out=sums[:, tt:tt + 1],
                        )
                        if tt % 2 == 1:
                            finish_slice(tt // 2)
                    done = new_done

            if a_last:
                emit_g_region()
                emit_a_region()
            else:
                emit_a_region()
                emit_g_region()

        # fi0 projection upfront; fi+1's projection threads into fi's attn
        # stream one instruction at a time.
        for fn in proj_steps(0):
            fn()
        for fi in range(FT):
            interleave = proj_steps(fi + 1) if fi + 1 < FT else []
            emit_attn(fi, interleave)

    nc.compile()
    return nc


def _prep_inputs(x, W_qkv):
    x = np.asarray(x, dtype=np.float32)
    W = np.asarray(W_qkv, dtype=np.float32)
    # per-fi W column blocks [K_fi (128) | Q_fi (128)], then packed
    # partition-major: w[p, ei, c] = wT[ei*128+p, c]
    wq = W[0:768].reshape(FT, 128, E)        # Q blocks per f-tile
    wk = W[768:1536].reshape(FT, 128, E)     # K blocks per f-tile
    wkq = np.stack([wk, wq], axis=1)         # [fi, kq, 128, e]
    wT = wkq.transpose(3, 0, 1, 2).reshape(E, 2 * 128 * FT)  # [e, cols]
    wP = wT.reshape(ET, 128, 2 * 128 * FT).transpose(1, 0, 2)  # [p, ei, cols]
    wA = np.ascontiguousarray(wP[:, :, 0:256].reshape(128, -1)).astype(np.float16)
    wB = np.ascontiguousarray(wP[:, :, 256:1536].reshape(128, -1)).astype(np.float16)
    in_maps = []
    for b in range(B):
        xT = x[b].T                           # [e, n]
        xP = np.ascontiguousarray(
            xT.reshape(ET, 128, N).transpose(1, 0, 2).reshape(128, -1)
        ).astype(np.float16)
        in_maps.append({"xP": xP, "wA": wA, "wB": wB})
    return in_maps


def _postprocess(res):
    outs = []
    inv = np.float32(1.0 / OUT_SCALE)
    for r in res.results:
        buf = r["out"]            # [48, 128, 2048] fp16, = out*1024
        buf = np.asarray(buf)
        if buf.dtype != np.float16:
            buf = buf.view(np.float16)
        full = buf.reshape(FT, 8, 128, 2, N).transpose(0, 3, 1, 2, 4)
        full = full.reshape(H, N, N).astype(np.float32) * inv
        outs.append(full)
    return np.stack(outs, axis=0)


def _run(x, W_qkv, trace=False):
    if "nc" not in _cache:
        _cache["nc"] = _build()
    nc = _cache["nc"]
    in_maps = _prep_inputs(x, W_qkv)
    res = run_bass_kernel_spmd(nc, in_maps, core_ids=list(range(B)), trace=trace)
    return _postprocess(res), res


def kernel(x, W_qkv):
    return _run(x, W_qkv)[0]


# revision 25
# speedup vs baseline: 1.0271x; 1.0012x over previous
"""Trainium2 Bass kernel for nn_Attention_layer_67877663146058.

Computes attn = softmax((x @ Wq.T) @ (x @ Wk.T)^T * hd**-0.5)
for x [8, 1024, 768], W_qkv [2304, 768] -> out [8, 12, 1024, 1024] fp32.
The V third of W_qkv never reaches the output and is not loaded.

Sharding: batch-parallel across the 8 NeuronCores (core b handles batch b,
all 12 heads).

Design notes (evolved from trace analysis of two prior versions):
- v1 was HBM-bound (107% avg HBM util) writing the 50MB fp32 output per
  core. All data is now fp16; the output is written as fp16 scaled by
  1024 (softmax entries down to ~2e-6 would be fp16 subnormals; the
  x1024 shift keeps them normal) and the host upcast multiplies by the
  exact power-of-two 1/1024. HBM traffic: 58MB -> 29MB per core.
- ACT exp is then the pacer. All DVE reduce-variant instructions
  (tensor_scalar+accum, tensor_reduce, tensor_tensor_reduce, bn_stats)
  run at 1x (~1.13us per [128,1024] tile) - only plain copy/scalar ops
  reach the 4x 2-byte mode (~0.49us). GpSimd cannot reduce along the
  free dim at all. So row sums are split:
    * 3 qb-slices per f-tile ("A"): exp in tile-aligned [128,1024] spans
      with the free ACT accumulator (costs +187ns READ_ACCUMULATOR and
      span-overhead fragmentation on ACT),
    * 5 qb-slices per f-tile ("G"): exp in [128,1536] spans (amortizes
      the ~250-cycle ACT per-instruction overhead), sums via the 1x DVE
      identity tensor_scalar(mult,add,accum_out).
- The normalize multiply (y * (1024/sum), fused two-op tensor_scalar,
  ~0.6us on DVE) mostly runs on GpSimd (~1.14us each, but the engine is
  otherwise idle); DVE keeps ~1 per f-tile plus the last f-tile's
  trailing A-slices (short tail). Engines land at roughly ACT 103 /
  DVE 100 / GpSimd 95 us busy - the three-way LP optimum for this
  instruction set.
- Input DMAs are consolidated into 4 fat partition-major transfers:
  each dma_start costs ~0.6us of serial descriptor-gen on the issuing
  sequencer, and 18 separate input DMAs delayed the first projection by
  ~9us in an earlier trace. Projection matmuls for f-tile fi+1 are
  threaded 1-2 instructions at a time between f-tile fi's score spans
  (an interleaved accumulation group only owns its PSUM bank), keeping
  the in-order PE stream from blocking the ACT-paced span pipeline.
- PSUM: 2x [128,512] proj slots + 2x [128,1536] score slots = 8 banks.
- A dependency-free exp at t=0 preloads the ACT spline table (~2.7us).
  The projection matmul stream itself warms the PE HAM clock gate.
"""

import numpy as np
from contextlib import ExitStack

import concourse.bacc as bacc
import concourse.mybir as mybir
import concourse.tile as tile

# bass_utils imports antenv.axon_hooks when BASS_TRACE is set in the
# environment; some images ship an antenv stub without that module. Register
# a no-op fallback so tracing degrades gracefully instead of crashing.
try:
    from antenv.axon_hooks import get_axon_ntff_profile_hook as _g  # noqa: F401
except Exception:
    import sys as _sys
    import types as _types

    _m = _types.ModuleType("antenv.axon_hooks")
    _state = {"h": None}
    _m.set_axon_ntff_profile_hook = lambda h: _state.__setitem__("h", h)
    _m.get_axon_ntff_profile_hook = lambda: _state["h"]
    _sys.modules["antenv.axon_hooks"] = _m
    try:
        import antenv as _antenv

        _antenv.axon_hooks = _m
    except Exception:
        pass

from concourse.bass_utils import run_bass_kernel_spmd

B = 8          # batches == cores
N = 1024       # tokens
E = 768        # embed dim
H = 12         # heads
HD = 64        # head dim
FT = 6         # f-tiles (2 heads per f-tile)
ET = E // 128  # 6 e-tiles
SPAN = 1536    # G-region exp span (one PSUM score slot, 3 banks)
SCALE = HD ** -0.5
OUT_SCALE = 1024.0   # fp16 output holds out*1024 to avoid subnormals

# qb slices per f-tile whose sums come from the ACT accumulator
A_QBS = (0, 3, 6)
# tile indices (2*slot+tt over the fi's emission order) whose normalize
# multiply runs on DVE; the rest run on GpSimd
DVE_MULT = frozenset({5})

_cache = {}


def _build():
    f32 = mybir.dt.float32
    f16 = mybir.dt.float16
    mult = mybir.AluOpType.mult
    add = mybir.AluOpType.add
    Exp = mybir.ActivationFunctionType.Exp
    nc = bacc.Bacc("TRN2", debug=False, num_devices=B)

    # inputs are packed partition-major on the host so each DMA moves
    # multi-KB contiguous runs per partition (128 fat descriptors/DMA)
    xP_d = nc.dram_tensor("xP", [128, ET * N], f16, kind="ExternalInput")
    wA_d = nc.dram_tensor("wA", [128, ET * 256], f16, kind="ExternalInput")
    wB_d = nc.dram_tensor("wB", [128, ET * 1280], f16, kind="ExternalInput")
    out_d = nc.dram_tensor("out", [FT * 8, 128, 2048], f16, kind="ExternalOutput")

    xP_src = xP_d.ap().rearrange("p (t n) -> p t n", t=ET)      # [128,6,1024]
    wA_src = wA_d.ap().rearrange("p (t c) -> p t c", t=ET)      # [128,6,256]
    wB_src = wB_d.ap().rearrange("p (t c) -> p t c", t=ET)      # [128,6,1280]
    out_ap = out_d.ap()

    with ExitStack() as ctx:
        tc = ctx.enter_context(tile.TileContext(nc))
        statics = ctx.enter_context(tc.tile_pool(name="statics", bufs=1))
        ypool = ctx.enter_context(tc.tile_pool(name="ypool", bufs=2))
        spool = ctx.enter_context(tc.tile_pool(name="spool", bufs=2))
        psum = ctx.enter_context(tc.tile_pool(name="psum", bufs=2, space="PSUM"))

        xt = statics.tile([128, ET, N], f16, tag="xt", name="xt")
        # W columns split into two contiguous tiles (f-tile 0 / rest) so
        # each input DMA moves multi-KB runs per partition
        wt0 = statics.tile([128, ET, 256], f16, tag="wt0", name="wt0")
        wtR = statics.tile([128, ET, 1280], f16, tag="wtR", name="wtR")
        # qkt[:, fi, 0, :] = K^T of f-tile fi, qkt[:, fi, 1, :] = Q^T
        qkt = statics.tile([128, FT, 2, N], f16, tag="qkt", name="qkt")

        def w_block(fi, kq, ei):
            if fi == 0:
                return wt0[:, ei, kq * 128:(kq + 1) * 128]
            c = (fi - 1) * 256 + kq * 128
            return wtR[:, ei, c:c + 128]

        # ACT table preload: dependency-free exp at t=0 pulls the ~2.7us
        # ACT_TABLE_LOAD off the critical path of the first real exp.
        warm = spool.tile([128, 1], f32, tag="warm", name="warm")
        nc.vector.memset(warm, 0.0)
        nc.scalar.activation(warm, warm, Exp)

        # Consolidated input loads (4 fat DMAs, 128 descriptors each): x
        # halves and f-tile-0 W columns first so the first projection can
        # start as early as possible.
        nc.sync.dma_start(xt[:, 0:3, :], xP_src[:, 0:3, :])
        nc.sync.dma_start(wt0, wA_src)
        nc.sync.dma_start(xt[:, 3:6, :], xP_src[:, 3:6, :])
        nc.sync.dma_start(wtR, wB_src)

        QUARTERS = [(0, 0), (0, 1), (1, 0), (1, 1)]  # K halves first

        def proj_steps(fi):
            # The projection for f-tile fi as a list of single-instruction
            # closures (6 accumulating matmuls + 1 evacuation cast per
            # quarter). Threading these one or two at a time between score
            # spans keeps the PE from blocking the ACT-paced span stream
            # for ~2us at a stretch (accumulation groups don't need to be
            # contiguous in the PE program; they only own their PSUM bank).
            steps = []
            for kq, half in QUARTERS:
                holder = []
                for ei in range(ET):
                    def mm_step(kq=kq, half=half, ei=ei, holder=holder):
                        if ei == 0:
                            holder.append(psum.tile(
                                [128, 512], f32, tag="pp",
                                name=f"pp{fi}_{kq}_{half}",
                            ))
                        nc.tensor.matmul(
                            holder[0],
                            lhsT=w_block(fi, kq, ei),
                            rhs=xt[:, ei, half * 512:(half + 1) * 512],
                            start=(ei == 0),
                            stop=(ei == ET - 1),
                            skip_group_check=True,
                        )
                    steps.append(mm_step)

                def cast_step(kq=kq, half=half, holder=holder):
                    nc.vector.tensor_copy(
                        qkt[:, fi, kq, half * 512:(half + 1) * 512], holder[0]
                    )
                steps.append(cast_step)
            return steps

        def score_mm(ps, off, fi, qb, hh, nh):
            lo = 64 * hh
            nc.tensor.matmul(
                ps[:, off:off + 512],
                lhsT=qkt[lo:lo + 64, fi, 1, qb * 128:(qb + 1) * 128],
                rhs=qkt[lo:lo + 64, fi, 0, nh * 512:(nh + 1) * 512],
                start=True,
                stop=True,
                tile_position=(lo, 0),
            )

        def emit_attn(fi, interleave):
            # y slot s (0..7) holds logical qb slot_qb[s]. A-slices (ACT
            # accumulator) lead so the G region is span-contiguous - except
            # in the last f-tile, where A-slices trail so the kernel tail
            # after the final exp is just recip+mult+DMA of one slice.
            a_last = fi == FT - 1
            g_qbs = [q for q in range(8) if q not in A_QBS]
            slot_qb = (g_qbs + list(A_QBS)) if a_last else (list(A_QBS) + g_qbs)
            a_slots = range(5, 8) if a_last else range(0, 3)
            g0 = 0 if a_last else len(A_QBS) * 2048
            y = ypool.tile([128, 16 * N], f16, tag="y", name=f"y{fi}")
            sums = spool.tile([128, 16], f32, tag="sums", name=f"sm{fi}")
            rec = spool.tile([128, 16], f32, tag="rec", name=f"rc{fi}")
            pending = list(interleave)

            def pull(k):
                for fn in pending[:k]:
                    fn()
                del pending[:k]

            def finish_slice(s):
                pull(1)
                qb = slot_qb[s]
                t0 = 2 * s
                nc.vector.reciprocal(rec[:, t0:t0 + 2], sums[:, t0:t0 + 2])
                for tt in (t0, t0 + 1):
                    yt = y[:, tt * N:(tt + 1) * N]
                    dve = (s >= 5) if a_last else (tt in DVE_MULT)
                    eng = nc.vector if dve else nc.gpsimd
                    eng.tensor_scalar(yt, yt, rec[:, tt:tt + 1], OUT_SCALE, mult, mult)
                nc.sync.dma_start(
                    out_ap[fi * 8 + qb], y[:, s * 2048:(s + 1) * 2048]
                )

            def emit_a_region():
                for s in a_slots:
                    qb = slot_qb[s]
                    for tt in (2 * s, 2 * s + 1):
                        hh = tt % 2
                        ps = psum.tile([128, SPAN], f32, tag="ps", name=f"psA{fi}_{tt}")
                        for nh in range(2):
                            score_mm(ps, nh * 512, fi, qb, hh, nh)
                        nc.scalar.activation(
                            y[:, tt * N:(tt + 1) * N], ps[:, 0:N], Exp, scale=SCALE,
                            accum_out=sums[:, tt:tt + 1],
                        )
                        pull(2)
                    finish_slice(s)

            def emit_g_region():
                done = g0
                for c0 in range(g0, g0 + len(g_qbs) * 2048, SPAN):
                    L = min(SPAN, g0 + len(g_qbs) * 2048 - c0)
                    ps = psum.tile([128, SPAN], f32, tag="ps", name=f"psG{fi}_{c0}")
                    for off in range(0, L, 512):
                        g = c0 + off
                        s = g // 2048
                        score_mm(ps, off, fi, slot_qb[s], (g // 1024) % 2, (g // 512) % 2)
                    nc.scalar.activation(
                        y[:, c0:c0 + L], ps[:, 0:L], Exp, scale=SCALE,
                    )
                    pull(2)
                    new_done = ((c0 + L) // N) * N
                    for tt in range(done // N, new_done // N):
                        yt = y[:, tt * N:(tt + 1) * N]
                        nc.vector.tensor_scalar(
                            yt, yt, 1.0, 0.0, mult, add, accum_out=sums[:, tt:tt + 1],
                        )
                        if tt % 2 == 1:
                            finish_slice(tt // 2)
                    done = new_done

            if a_last:
                emit_g_region()
                emit_a_region()
            else:
                emit_a_region()
                emit_g_region()

        # fi0 projection upfront; fi+1's projection threads into fi's attn
        # stream one instruction at a time.
        for fn in proj_steps(0):
            fn()
        for fi in range(FT):
            interleave = proj_steps(fi + 1) if fi + 1 < FT else []
            emit_attn(fi, interleave)

    nc.compile()
    return nc


def _prep_inputs(x, W_qkv):
    x = np.asarray(x, dtype=np.float32)
    W = np.asarray(W_qkv, dtype=np.float32)
    # per-fi W column blocks [K_fi (128) | Q_fi (128)], then packed
    # partition-major: w[p, ei, c] = wT[ei*128+p, c]
    wq = W[0:768].reshape(FT, 128, E)        # Q blocks per f-tile
    wk = W[768:1536].reshape(FT, 128, E)     # K blocks per f-tile
    wkq = np.stack([wk, wq], axis=1)         # [fi, kq, 128, e]
    wT = wkq.transpose(3, 0, 1, 2).reshape(E, 2 * 128 * FT)  # [e, cols]
    wP = wT.reshape(ET, 128, 2 * 128 * FT).transpose(1, 0, 2)  # [p, ei, cols]
    wA = np.ascontiguousarray(wP[:, :, 0:256].reshape(128, -1)).astype(np.float16)
    wB = np.ascontiguousarray(wP[:, :, 256:1536].reshape(128, -1)).astype(np.float16)
    in_maps = []
    for b in range(B):
        xT = x[b].T                           # [e, n]
        xP = np.ascontiguousarray(
            xT.reshape(ET, 128, N).transpose(1, 0, 2).reshape(128, -1)
        ).astype(np.float16)
        in_maps.append({"xP": xP, "wA": wA, "wB": wB})
    return in_maps


def _postprocess(res):
    outs = []
    inv = np.float32(1.0 / OUT_SCALE)
    for r in res.results:
        buf = r["out"]            # [48, 128, 2048] fp16, = out*1024
        buf = np.asarray(buf)
        if buf.dtype != np.float16:
            buf = buf.view(np.float16)
        full = buf.reshape(FT, 8, 128, 2, N).transpose(0, 3, 1, 2, 4)
        full = full.reshape(H, N, N).astype(np.float32) * inv
        outs.append(full)
    return np.stack(outs, axis=0)


def _run(x, W_qkv, trace=False):
    if "nc" not in _cache:
        _cache["nc"] = _build()
    nc = _cache["nc"]
    in_maps = _prep_inputs(x, W_qkv)
    res = run_bass_kernel_spmd(nc, in_maps, core_ids=list(range(B)), trace=trace)
    return _postprocess(res), res


def kernel(x, W_qkv):
    return _run(x, W_qkv)[0]


# revision 26
# speedup vs baseline: 1.0305x; 1.0033x over previous
"""Trainium2 Bass kernel for nn_Attention_layer_67877663146058.

Computes attn = softmax((x @ Wq.T) @ (x @ Wk.T)^T * hd**-0.5)
for x [8, 1024, 768], W_qkv [2304, 768] -> out [8, 12, 1024, 1024] fp32.
The V third of W_qkv never reaches the output and is not loaded.

Sharding: batch-parallel across the 8 NeuronCores (core b handles batch b,
all 12 heads).

Design notes (evolved from trace analysis of two prior versions):
- v1 was HBM-bound (107% avg HBM util) writing the 50MB fp32 output per
  core. All data is now fp16; the output is written as fp16 scaled by
  1024 (softmax entries down to ~2e-6 would be fp16 subnormals; the
  x1024 shift keeps them normal) and the host upcast multiplies by the
  exact power-of-two 1/1024. HBM traffic: 58MB -> 29MB per core.
- ACT exp is then the pacer. All DVE reduce-variant instructions
  (tensor_scalar+accum, tensor_reduce, tensor_tensor_reduce, bn_stats)
  run at 1x (~1.13us per [128,1024] tile) - only plain copy/scalar ops
  reach the 4x 2-byte mode (~0.49us). GpSimd cannot reduce along the
  free dim at all. So row sums are split:
    * 3 qb-slices per f-tile ("A"): exp in tile-aligned [128,1024] spans
      with the free ACT accumulator (costs +187ns READ_ACCUMULATOR and
      span-overhead fragmentation on ACT),
    * 5 qb-slices per f-tile ("G"): exp in [128,1536] spans (amortizes
      the ~250-cycle ACT per-instruction overhead), sums via the 1x DVE
      identity tensor_scalar(mult,add,accum_out).
- The normalize multiply (y * (1024/sum), fused two-op tensor_scalar,
  ~0.6us on DVE) mostly runs on GpSimd (~1.14us each, but the engine is
  otherwise idle); DVE keeps ~1 per f-tile plus the last f-tile's
  trailing A-slices (short tail). Engines land at roughly ACT 103 /
  DVE 100 / GpSimd 95 us busy - the three-way LP optimum for this
  instruction set.
- Input DMAs are consolidated into 4 fat partition-major transfers:
  each dma_start costs ~0.6us of serial descriptor-gen on the issuing
  sequencer, and 18 separate input DMAs delayed the first projection by
  ~9us in an earlier trace. Projection matmuls for f-tile fi+1 are
  threaded 1-2 instructions at a time between f-tile fi's score spans
  (an interleaved accumulation group only owns its PSUM bank), keeping
  the in-order PE stream from blocking the ACT-paced span pipeline.
- PSUM: 2x [128,512] proj slots + 2x [128,1536] score slots = 8 banks.
- A dependency-free exp at t=0 preloads the ACT spline table (~2.7us).
  The projection matmul stream itself warms the PE HAM clock gate.
"""

import numpy as np
from contextlib import ExitStack

import concourse.bacc as bacc
import concourse.mybir as mybir
import concourse.tile as tile

# bass_utils imports antenv.axon_hooks when BASS_TRACE is set in the
# environment; some images ship an antenv stub without that module. Register
# a no-op fallback so tracing degrades gracefully instead of crashing.
try:
    from antenv.axon_hooks import get_axon_ntff_profile_hook as _g  # noqa: F401
except Exception:
    import sys as _sys
    import types as _types

    _m = _types.ModuleType("antenv.axon_hooks")
    _state = {"h": None}
    _m.set_axon_ntff_profile_hook = lambda h: _state.__setitem__("h", h)
    _m.get_axon_ntff_profile_hook = lambda: _state["h"]
    _sys.modules["antenv.axon_hooks"] = _m
    try:
        import antenv as _antenv

        _antenv.axon_hooks = _m
    except Exception:
        pass

from concourse.bass_utils import run_bass_kernel_spmd

B = 8          # batches == cores
N = 1024       # tokens
E = 768        # embed dim
H = 12         # heads
HD = 64        # head dim
FT = 6         # f-tiles (2 heads per f-tile)
ET = E // 128  # 6 e-tiles
SPAN = 1536    # G-region exp span (one PSUM score slot, 3 banks)
SCALE = HD ** -0.5
OUT_SCALE = 1024.0   # fp16 output holds out*1024 to avoid subnormals

# qb slices per f-tile whose sums come from the ACT accumulator
A_QBS = (0, 3, 6)
# tile indices (2*slot+tt over the fi's emission order) whose normalize
# multiply runs on DVE; the rest run on GpSimd
DVE_MULT = frozenset({5})

_cache = {}


def _build():
    f32 = mybir.dt.float32
    f16 = mybir.dt.float16
    mult = mybir.AluOpType.mult
    add = mybir.AluOpType.add
    Exp = mybir.ActivationFunctionType.Exp
    nc = bacc.Bacc("TRN2", debug=False, num_devices=B)

    # inputs are packed partition-major on the host so each DMA moves
    # multi-KB contiguous runs per partition (128 fat descriptors/DMA)
    xP_d = nc.dram_tensor("xP", [128, ET * N], f16, kind="ExternalInput")
    wA_d = nc.dram_tensor("wA", [128, ET * 256], f16, kind="ExternalInput")
    wB_d = nc.dram_tensor("wB", [128, ET * 1280], f16, kind="ExternalInput")
    out_d = nc.dram_tensor("out", [FT * 8, 128, 2048], f16, kind="ExternalOutput")

    xP_src = xP_d.ap().rearrange("p (t n) -> p t n", t=ET)      # [128,6,1024]
    wA_src = wA_d.ap().rearrange("p (t c) -> p t c", t=ET)      # [128,6,256]
    wB_src = wB_d.ap().rearrange("p (t c) -> p t c", t=ET)      # [128,6,1280]
    out_ap = out_d.ap()

    with ExitStack() as ctx:
        tc = ctx.enter_context(tile.TileContext(nc))
        statics = ctx.enter_context(tc.tile_pool(name="statics", bufs=1))
        ypool = ctx.enter_context(tc.tile_pool(name="ypool", bufs=2))
        spool = ctx.enter_context(tc.tile_pool(name="spool", bufs=2))
        psum = ctx.enter_context(tc.tile_pool(name="psum", bufs=2, space="PSUM"))

        xt = statics.tile([128, ET, N], f16, tag="xt", name="xt")
        # W columns split into two contiguous tiles (f-tile 0 / rest) so
        # each input DMA moves multi-KB runs per partition
        wt0 = statics.tile([128, ET, 256], f16, tag="wt0", name="wt0")
        wtR = statics.tile([128, ET, 1280], f16, tag="wtR", name="wtR")
        # qkt[:, fi, 0, :] = K^T of f-tile fi, qkt[:, fi, 1, :] = Q^T
        qkt = statics.tile([128, FT, 2, N], f16, tag="qkt", name="qkt")

        def w_block(fi, kq, ei):
            if fi == 0:
                return wt0[:, ei, kq * 128:(kq + 1) * 128]
            c = (fi - 1) * 256 + kq * 128
            return wtR[:, ei, c:c + 128]

        # ACT table preload: dependency-free exp at t=0 pulls the ~2.7us
        # ACT_TABLE_LOAD off the critical path of the first real exp.
        warm = spool.tile([128, 1], f32, tag="warm", name="warm")
        nc.vector.memset(warm, 0.0)
        nc.scalar.activation(warm, warm, Exp)

        # Consolidated input loads (4 fat DMAs, 128 descriptors each): x
        # halves and f-tile-0 W columns first so the first projection can
        # start as early as possible.
        nc.sync.dma_start(xt[:, 0:3, :], xP_src[:, 0:3, :])
        nc.sync.dma_start(wt0, wA_src)
        nc.sync.dma_start(xt[:, 3:6, :], xP_src[:, 3:6, :])
        nc.sync.dma_start(wtR, wB_src)

        QUARTERS = [(0, 0), (0, 1), (1, 0), (1, 1)]  # K halves first

        def proj_steps(fi):
            # The projection for f-tile fi as a list of single-instruction
            # closures (6 accumulating matmuls + 1 evacuation cast per
            # quarter). Threading these one or two at a time between score
            # spans keeps the PE from blocking the ACT-paced span stream
            # for ~2us at a stretch (accumulation groups don't need to be
            # contiguous in the PE program; they only own their PSUM bank).
            steps = []
            for kq, half in QUARTERS:
                holder = []
                for ei in range(ET):
                    def mm_step(kq=kq, half=half, ei=ei, holder=holder):
                        if ei == 0:
                            holder.append(psum.tile(
                                [128, 512], f32, tag="pp",
                                name=f"pp{fi}_{kq}_{half}",
                            ))
                        nc.tensor.matmul(
                            holder[0],
                            lhsT=w_block(fi, kq, ei),
                            rhs=xt[:, ei, half * 512:(half + 1) * 512],
                            start=(ei == 0),
                            stop=(ei == ET - 1),
                            skip_group_check=True,
                        )
                    steps.append(mm_step)

                def cast_step(kq=kq, half=half, holder=holder):
                    nc.vector.tensor_copy(
                        qkt[:, fi, kq, half * 512:(half + 1) * 512], holder[0]
                    )
                steps.append(cast_step)
            return steps

        def score_mm(ps, off, fi, qb, hh, nh):
            lo = 64 * hh
            nc.tensor.matmul(
                ps[:, off:off + 512],
                lhsT=qkt[lo:lo + 64, fi, 1, qb * 128:(qb + 1) * 128],
                rhs=qkt[lo:lo + 64, fi, 0, nh * 512:(nh + 1) * 512],
                start=True,
                stop=True,
                tile_position=(lo, 0),
            )

        def emit_attn(fi, interleave):
            # y slot s (0..7) holds logical qb slot_qb[s]. A-slices (ACT
            # accumulator, self-contained 1024-col spans) are interleaved
            # BETWEEN span-aligned G-runs (3 slices = 4x1536, 2 slices =
            # 2x1536+1024): the G-region generates ~1.6x more DVE work per
            # ACT span than ACT consumes (identity sums + recip + mult +
            # proj cast), so a contiguous G-region lets the DVE queue grow
            # ~6us deep and the projection cast stuck in it stalls the
            # pp-ring -> PE -> ACT. A-slices underload DVE and drain it.
            # In the last f-tile the A-slices trail so the tail after the
            # final exp is just recip+mult+DMA.
            a_last = fi == FT - 1
            a_qbs = list(A_QBS)
            g_qbs = [q for q in range(8) if q not in A_QBS]
            if a_last:
                segs = [("G", g_qbs[0:3]), ("A", [a_qbs[0]]),
                        ("G", g_qbs[3:5]), ("A", [a_qbs[1]]), ("A", [a_qbs[2]])]
            else:
                segs = [("A", [a_qbs[0]]), ("G", g_qbs[0:3]),
                        ("A", [a_qbs[1]]), ("G", g_qbs[3:5]), ("A", [a_qbs[2]])]
            slot_qb = [qb for _, qbs in segs for qb in qbs]
            y = ypool.tile([128, 16 * N], f16, tag="y", name=f"y{fi}")
            sums = spool.tile([128, 16], f32, tag="sums", name=f"sm{fi}")
            rec = spool.tile([128, 16], f32, tag="rec", name=f"rc{fi}")
            pending = list(interleave)

            def pull(k):
                for fn in pending[:k]:
                    fn()
                del pending[:k]

            def finish_slice(s):
                pull(1)
                qb = slot_qb[s]
                t0 = 2 * s
                nc.vector.reciprocal(rec[:, t0:t0 + 2], sums[:, t0:t0 + 2])
                for tt in (t0, t0 + 1):
                    yt = y[:, tt * N:(tt + 1) * N]
                    dve = (s >= 5) if a_last else (tt in DVE_MULT)
                    eng = nc.vector if dve else nc.gpsimd
                    eng.tensor_scalar(yt, yt, rec[:, tt:tt + 1], OUT_SCALE, mult, mult)
                nc.sync.dma_start(
                    out_ap[fi * 8 + qb], y[:, s * 2048:(s + 1) * 2048]
                )

            s = 0
            for kind, qbs in segs:
                if kind == "A":
                    qb = qbs[0]
                    for tt in (2 * s, 2 * s + 1):
                        hh = tt % 2
                        ps = psum.tile([128, SPAN], f32, tag="ps", name=f"psA{fi}_{tt}")
                        for nh in range(2):
                            score_mm(ps, nh * 512, fi, qb, hh, nh)
                        nc.scalar.activation(
                            y[:, tt * N:(tt + 1) * N], ps[:, 0:N], Exp, scale=SCALE,
                            accum_out=sums[:, tt:tt + 1],
                        )
                        pull(2)
                    finish_slice(s)
                    s += 1
                else:
                    run0 = s * 2048
                    run1 = run0 + len(qbs) * 2048
                    done = run0
                    for c0 in range(run0, run1, SPAN):
                        L = min(SPAN, run1 - c0)
                        ps = psum.tile([128, SPAN], f32, tag="ps", name=f"psG{fi}_{c0}")
                        for off in range(0, L, 512):
                            g = c0 + off
                            sg = g // 2048
                            score_mm(ps, off, fi, slot_qb[sg],
                                     (g // 1024) % 2, (g // 512) % 2)
                        nc.scalar.activation(
                            y[:, c0:c0 + L], ps[:, 0:L], Exp, scale=SCALE,
                        )
                        pull(2)
                        new_done = ((c0 + L) // N) * N
                        for tt in range(done // N, new_done // N):
                            yt = y[:, tt * N:(tt + 1) * N]
                            nc.vector.tensor_scalar(
                                yt, yt, 1.0, 0.0, mult, add,
                                accum_out=sums[:, tt:tt + 1],
                            )
                            if tt % 2 == 1:
                                finish_slice(tt // 2)
                        done = new_done
                    s += len(qbs)

        # fi0 projection upfront; fi+1's projection threads into fi's attn
        # stream one instruction at a time.
        for fn in proj_steps(0):
            fn()
        for fi in range(FT):
            interleave = proj_steps(fi + 1) if fi + 1 < FT else []
            emit_attn(fi, interleave)

    nc.compile()
    return nc


def _prep_inputs(x, W_qkv):
    x = np.asarray(x, dtype=np.float32)
    W = np.asarray(W_qkv, dtype=np.float32)
    # per-fi W column blocks [K_fi (128) | Q_fi (128)], then packed
    # partition-major: w[p, ei, c] = wT[ei*128+p, c]
    wq = W[0:768].reshape(FT, 128, E)        # Q blocks per f-tile
    wk = W[768:1536].reshape(FT, 128, E)     # K blocks per f-tile
    wkq = np.stack([wk, wq], axis=1)         # [fi, kq, 128, e]
    wT = wkq.transpose(3, 0, 1, 2).reshape(E, 2 * 128 * FT)  # [e, cols]
    wP = wT.reshape(ET, 128, 2 * 128 * FT).transpose(1, 0, 2)  # [p, ei, cols]
    wA = np.ascontiguousarray(wP[:, :, 0:256].reshape(128, -1)).astype(np.float16)
    wB = np.ascontiguousarray(wP[:, :, 256:1536].reshape(128, -1)).astype(np.float16)
    in_maps = []
    for b in range(B):
        xT = x[b].T                           # [e, n]
        xP = np.ascontiguousarray(
            xT.reshape(ET, 128, N).transpose(1, 0, 2).reshape(128, -1)
        ).astype(np.float16)
        in_maps.append({"xP": xP, "wA": wA, "wB": wB})
    return in_maps


def _postprocess(res):
    outs = []
    inv = np.float32(1.0 / OUT_SCALE)
    for r in res.results:
        buf = r["out"]            # [48, 128, 2048] fp16, = out*1024
        buf = np.asarray(buf)
        if buf.dtype != np.float16:
            buf = buf.view(np.float16)
        full = buf.reshape(FT, 8, 128, 2, N).transpose(0, 3, 1, 2, 4)
        full = full.reshape(H, N, N).astype(np.float32) * inv
        outs.append(full)
    return np.stack(outs, axis=0)


def _run(x, W_qkv, trace=False):
    if "nc" not in _cache:
        _cache["nc"] = _build()
    nc = _cache["nc"]
    in_maps = _prep_inputs(x, W_qkv)
    res = run_bass_kernel_spmd(nc, in_maps, core_ids=list(range(B)), trace=trace)
    return _postprocess(res), res


def kernel(x, W_qkv):
    return _run(x, W_qkv)[0]


# revision 27
# speedup vs baseline: 1.0702x; 1.0385x over previous
"""Trainium2 Bass kernel for nn_Attention_layer_67877663146058.

Computes attn = softmax((x @ Wq.T) @ (x @ Wk.T)^T * hd**-0.5)
for x [8, 1024, 768], W_qkv [2304, 768] -> out [8, 12, 1024, 1024] fp32.
The V third of W_qkv never reaches the output and is not loaded.

Sharding: batch-parallel across the 8 NeuronCores (core b handles batch b,
all 12 heads).

Design notes (evolved from trace analysis of two prior versions):
- v1 was HBM-bound (107% avg HBM util) writing the 50MB fp32 output per
  core. All data is now fp16; the output is written as fp16 scaled by
  1024 (softmax entries down to ~2e-6 would be fp16 subnormals; the
  x1024 shift keeps them normal) and the host upcast multiplies by the
  exact power-of-two 1/1024. HBM traffic: 58MB -> 29MB per core.
- ACT exp is then the pacer. All DVE reduce-variant instructions
  (tensor_scalar+accum, tensor_reduce, tensor_tensor_reduce, bn_stats)
  run at 1x (~1.13us per [128,1024] tile) - only plain copy/scalar ops
  reach the 4x 2-byte mode (~0.49us). GpSimd cannot reduce along the
  free dim at all. So row sums are split:
    * 3 qb-slices per f-tile ("A"): exp in tile-aligned [128,1024] spans
      with the free ACT accumulator (costs +187ns READ_ACCUMULATOR and
      span-overhead fragmentation on ACT),
    * 5 qb-slices per f-tile ("G"): exp in [128,1536] spans (amortizes
      the ~250-cycle ACT per-instruction overhead), sums via the 1x DVE
      identity tensor_scalar(mult,add,accum_out).
- The normalize multiply (y * (1024/sum), fused two-op tensor_scalar,
  ~0.6us on DVE) mostly runs on GpSimd (~1.14us each, but the engine is
  otherwise idle); DVE keeps ~1 per f-tile plus the last f-tile's
  trailing A-slices (short tail). Engines land at roughly ACT 103 /
  DVE 100 / GpSimd 95 us busy - the three-way LP optimum for this
  instruction set.
- Input DMAs are consolidated into 4 fat partition-major transfers:
  each dma_start costs ~0.6us of serial descriptor-gen on the issuing
  sequencer, and 18 separate input DMAs delayed the first projection by
  ~9us in an earlier trace. Projection matmuls for f-tile fi+1 are
  threaded 1-2 instructions at a time between f-tile fi's score spans
  (an interleaved accumulation group only owns its PSUM bank), keeping
  the in-order PE stream from blocking the ACT-paced span pipeline.
- PSUM: 2x [128,512] proj slots + 2x [128,1536] score slots = 8 banks.
- A dependency-free exp at t=0 preloads the ACT spline table (~2.7us).
  The projection matmul stream itself warms the PE HAM clock gate.
"""

import numpy as np
from contextlib import ExitStack

import concourse.bacc as bacc
import concourse.mybir as mybir
import concourse.tile as tile

# bass_utils imports antenv.axon_hooks when BASS_TRACE is set in the
# environment; some images ship an antenv stub without that module. Register
# a no-op fallback so tracing degrades gracefully instead of crashing.
try:
    from antenv.axon_hooks import get_axon_ntff_profile_hook as _g  # noqa: F401
except Exception:
    import sys as _sys
    import types as _types

    _m = _types.ModuleType("antenv.axon_hooks")
    _state = {"h": None}
    _m.set_axon_ntff_profile_hook = lambda h: _state.__setitem__("h", h)
    _m.get_axon_ntff_profile_hook = lambda: _state["h"]
    _sys.modules["antenv.axon_hooks"] = _m
    try:
        import antenv as _antenv

        _antenv.axon_hooks = _m
    except Exception:
        pass

from concourse.bass_utils import run_bass_kernel_spmd

B = 8          # batches == cores
N = 1024       # tokens
E = 768        # embed dim
H = 12         # heads
HD = 64        # head dim
FT = 6         # f-tiles (2 heads per f-tile)
ET = E // 128  # 6 e-tiles
SPAN = 1536    # G-region exp span (one PSUM score slot, 3 banks)
SCALE = HD ** -0.5
OUT_SCALE = 1024.0   # fp16 output holds out*1024 to avoid subnormals

# qb slices per f-tile whose sums come from the ACT accumulator
A_QBS = (0, 3, 6)
# tile indices (2*slot+tt over the fi's emission order) whose normalize
# multiply runs on DVE; the rest run on GpSimd
DVE_MULT = frozenset({5, 11})

_cache = {}


def _build():
    f32 = mybir.dt.float32
    f16 = mybir.dt.float16
    mult = mybir.AluOpType.mult
    add = mybir.AluOpType.add
    Exp = mybir.ActivationFunctionType.Exp
    nc = bacc.Bacc("TRN2", debug=False, num_devices=B)

    # inputs are packed partition-major on the host so each DMA moves
    # multi-KB contiguous runs per partition (128 fat descriptors/DMA)
    xP_d = nc.dram_tensor("xP", [128, ET * N], f16, kind="ExternalInput")
    wA_d = nc.dram_tensor("wA", [128, ET * 256], f16, kind="ExternalInput")
    wB_d = nc.dram_tensor("wB", [128, ET * 1280], f16, kind="ExternalInput")
    out_d = nc.dram_tensor("out", [FT * 8, 128, 2048], f16, kind="ExternalOutput")

    xP_src = xP_d.ap().rearrange("p (t n) -> p t n", t=ET)      # [128,6,1024]
    wA_src = wA_d.ap().rearrange("p (t c) -> p t c", t=ET)      # [128,6,256]
    wB_src = wB_d.ap().rearrange("p (t c) -> p t c", t=ET)      # [128,6,1280]
    out_ap = out_d.ap()

    with ExitStack() as ctx:
        tc = ctx.enter_context(tile.TileContext(nc))
        statics = ctx.enter_context(tc.tile_pool(name="statics", bufs=1))
        ypool = ctx.enter_context(tc.tile_pool(name="ypool", bufs=2))
        spool = ctx.enter_context(tc.tile_pool(name="spool", bufs=2))
        psum = ctx.enter_context(tc.tile_pool(name="psum", bufs=2, space="PSUM"))

        xt = statics.tile([128, ET, N], f16, tag="xt", name="xt")
        # W columns split into two contiguous tiles (f-tile 0 / rest) so
        # each input DMA moves multi-KB runs per partition
        wt0 = statics.tile([128, ET, 256], f16, tag="wt0", name="wt0")
        wtR = statics.tile([128, ET, 1280], f16, tag="wtR", name="wtR")
        # qkt[:, fi, 0, :] = K^T of f-tile fi, qkt[:, fi, 1, :] = Q^T
        qkt = statics.tile([128, FT, 2, N], f16, tag="qkt", name="qkt")

        def w_block(fi, kq, ei):
            if fi == 0:
                return wt0[:, ei, kq * 128:(kq + 1) * 128]
            c = (fi - 1) * 256 + kq * 128
            return wtR[:, ei, c:c + 128]

        # ACT table preload: dependency-free exp at t=0 pulls the ~2.7us
        # ACT_TABLE_LOAD off the critical path of the first real exp.
        warm = spool.tile([128, 1], f32, tag="warm", name="warm")
        nc.vector.memset(warm, 0.0)
        nc.scalar.activation(warm, warm, Exp)

        # Consolidated input loads (4 fat DMAs, 128 descriptors each): x
        # halves and f-tile-0 W columns first so the first projection can
        # start as early as possible.
        nc.sync.dma_start(xt[:, 0:3, :], xP_src[:, 0:3, :])
        nc.sync.dma_start(wt0, wA_src)
        nc.sync.dma_start(xt[:, 3:6, :], xP_src[:, 3:6, :])
        nc.sync.dma_start(wtR, wB_src)

        QUARTERS = [(0, 0), (0, 1), (1, 0), (1, 1)]  # K halves first

        def proj_steps(fi):
            # The projection for f-tile fi as a list of single-instruction
            # closures (6 accumulating matmuls + 1 evacuation cast per
            # quarter). Threading these one or two at a time between score
            # spans keeps the PE from blocking the ACT-paced span stream
            # for ~2us at a stretch (accumulation groups don't need to be
            # contiguous in the PE program; they only own their PSUM bank).
            steps = []
            for qi, (kq, half) in enumerate(QUARTERS):
                # f-tile 0 runs during the fill phase while the score slots
                # are idle: its Q quarters borrow "ps" slots so all four
                # quarters proceed without pp-ring (cast) waits.
                tag = "ps" if (fi == 0 and qi >= 2) else "pp"
                shape = [128, SPAN] if tag == "ps" else [128, 512]
                holder = []
                for ei in range(ET):
                    def mm_step(kq=kq, half=half, ei=ei, holder=holder,
                                tag=tag, shape=shape):
                        if ei == 0:
                            holder.append(psum.tile(
                                shape, f32, tag=tag,
                                name=f"pp{fi}_{kq}_{half}",
                            ))
                        nc.tensor.matmul(
                            holder[0][:, 0:512],
                            lhsT=w_block(fi, kq, ei),
                            rhs=xt[:, ei, half * 512:(half + 1) * 512],
                            start=(ei == 0),
                            stop=(ei == ET - 1),
                            skip_group_check=True,
                        )
                    steps.append(mm_step)

                def cast_step(kq=kq, half=half, holder=holder):
                    nc.vector.tensor_copy(
                        qkt[:, fi, kq, half * 512:(half + 1) * 512],
                        holder[0][:, 0:512],
                    )
                steps.append(cast_step)
            return steps

        def score_mm(ps, off, fi, qb, hh, nh):
            lo = 64 * hh
            nc.tensor.matmul(
                ps[:, off:off + 512],
                lhsT=qkt[lo:lo + 64, fi, 1, qb * 128:(qb + 1) * 128],
                rhs=qkt[lo:lo + 64, fi, 0, nh * 512:(nh + 1) * 512],
                start=True,
                stop=True,
                tile_position=(lo, 0),
            )

        def emit_attn(fi, interleave):
            # y slot s (0..7) holds logical qb slot_qb[s]. A-slices (ACT
            # accumulator, self-contained 1024-col spans) are interleaved
            # BETWEEN span-aligned G-runs (3 slices = 4x1536, 2 slices =
            # 2x1536+1024): the G-region generates ~1.6x more DVE work per
            # ACT span than ACT consumes (identity sums + recip + mult +
            # proj cast), so a contiguous G-region lets the DVE queue grow
            # ~6us deep and the projection cast stuck in it stalls the
            # pp-ring -> PE -> ACT. A-slices underload DVE and drain it.
            # In the last f-tile the A-slices trail so the tail after the
            # final exp is just recip+mult+DMA.
            a_last = fi == FT - 1
            a_qbs = list(A_QBS)
            g_qbs = [q for q in range(8) if q not in A_QBS]
            if a_last:
                segs = [("G", g_qbs[0:3]), ("A", [a_qbs[0]]),
                        ("G", g_qbs[3:5]), ("A", [a_qbs[1]]), ("A", [a_qbs[2]])]
            else:
                segs = [("A", [a_qbs[0]]), ("G", g_qbs[0:3]),
                        ("A", [a_qbs[1]]), ("G", g_qbs[3:5]), ("A", [a_qbs[2]])]
            slot_qb = [qb for _, qbs in segs for qb in qbs]
            y = ypool.tile([128, 16 * N], f16, tag="y", name=f"y{fi}")
            sums = spool.tile([128, 16], f32, tag="sums", name=f"sm{fi}")
            rec = spool.tile([128, 16], f32, tag="rec", name=f"rc{fi}")
            pending = list(interleave)

            def pull(k):
                for fn in pending[:k]:
                    fn()
                del pending[:k]

            def finish_slice(s):
                pull(1)
                qb = slot_qb[s]
                t0 = 2 * s
                nc.vector.reciprocal(rec[:, t0:t0 + 2], sums[:, t0:t0 + 2])
                for tt in (t0, t0 + 1):
                    yt = y[:, tt * N:(tt + 1) * N]
                    dve = (s >= 5) if a_last else (tt in DVE_MULT)
                    eng = nc.vector if dve else nc.gpsimd
                    eng.tensor_scalar(yt, yt, rec[:, tt:tt + 1], OUT_SCALE, mult, mult)
                nc.sync.dma_start(
                    out_ap[fi * 8 + qb], y[:, s * 2048:(s + 1) * 2048]
                )

            s = 0
            for kind, qbs in segs:
                if kind == "A":
                    qb = qbs[0]
                    for tt in (2 * s, 2 * s + 1):
                        hh = tt % 2
                        ps = psum.tile([128, SPAN], f32, tag="ps", name=f"psA{fi}_{tt}")
                        for nh in range(2):
                            score_mm(ps, nh * 512, fi, qb, hh, nh)
                        nc.scalar.activation(
                            y[:, tt * N:(tt + 1) * N], ps[:, 0:N], Exp, scale=SCALE,
                            accum_out=sums[:, tt:tt + 1],
                        )
                        pull(2)
                    finish_slice(s)
                    s += 1
                else:
                    run0 = s * 2048
                    run1 = run0 + len(qbs) * 2048
                    done = run0
                    for c0 in range(run0, run1, SPAN):
                        L = min(SPAN, run1 - c0)
                        ps = psum.tile([128, SPAN], f32, tag="ps", name=f"psG{fi}_{c0}")
                        for off in range(0, L, 512):
                            g = c0 + off
                            sg = g // 2048
                            score_mm(ps, off, fi, slot_qb[sg],
                                     (g // 1024) % 2, (g // 512) % 2)
                        nc.scalar.activation(
                            y[:, c0:c0 + L], ps[:, 0:L], Exp, scale=SCALE,
                        )
                        pull(2)
                        new_done = ((c0 + L) // N) * N
                        for tt in range(done // N, new_done // N):
                            yt = y[:, tt * N:(tt + 1) * N]
                            nc.vector.tensor_scalar(
                                yt, yt, 1.0, 0.0, mult, add,
                                accum_out=sums[:, tt:tt + 1],
                            )
                            if tt % 2 == 1:
                                finish_slice(tt // 2)
                        done = new_done
                    s += len(qbs)

        # fi0 projection upfront; fi+1's projection threads into fi's attn
        # stream one instruction at a time.
        for fn in proj_steps(0):
            fn()
        for fi in range(FT):
            interleave = proj_steps(fi + 1) if fi + 1 < FT else []
            emit_attn(fi, interleave)

    nc.compile()
    return nc


def _prep_inputs(x, W_qkv):
    x = np.asarray(x, dtype=np.float32)
    W = np.asarray(W_qkv, dtype=np.float32)
    # per-fi W column blocks [K_fi (128) | Q_fi (128)], then packed
    # partition-major: w[p, ei, c] = wT[ei*128+p, c]
    wq = W[0:768].reshape(FT, 128, E)        # Q blocks per f-tile
    wk = W[768:1536].reshape(FT, 128, E)     # K blocks per f-tile
    wkq = np.stack([wk, wq], axis=1)         # [fi, kq, 128, e]
    wT = wkq.transpose(3, 0, 1, 2).reshape(E, 2 * 128 * FT)  # [e, cols]
    wP = wT.reshape(ET, 128, 2 * 128 * FT).transpose(1, 0, 2)  # [p, ei, cols]
    wA = np.ascontiguousarray(wP[:, :, 0:256].reshape(128, -1)).astype(np.float16)
    wB = np.ascontiguousarray(wP[:, :, 256:1536].reshape(128, -1)).astype(np.float16)
    in_maps = []
    for b in range(B):
        xT = x[b].T                           # [e, n]
        xP = np.ascontiguousarray(
            xT.reshape(ET, 128, N).transpose(1, 0, 2).reshape(128, -1)
        ).astype(np.float16)
        in_maps.append({"xP": xP, "wA": wA, "wB": wB})
    return in_maps


def _postprocess(res):
    outs = []
    inv = np.float32(1.0 / OUT_SCALE)
    for r in res.results:
        buf = r["out"]            # [48, 128, 2048] fp16, = out*1024
        buf = np.asarray(buf)
        if buf.dtype != np.float16:
            buf = buf.view(np.float16)
        full = buf.reshape(FT, 8, 128, 2, N).transpose(0, 3, 1, 2, 4)
        full = full.reshape(H, N, N).astype(np.float32) * inv
        outs.append(full)
    return np.stack(outs, axis=0)


def _run(x, W_qkv, trace=False):
    if "nc" not in _cache:
        _cache["nc"] = _build()
    nc = _cache["nc"]
    in_maps = _prep_inputs(x, W_qkv)
    res = run_bass_kernel_spmd(nc, in_maps, core_ids=list(range(B)), trace=trace)
    return _postprocess(res), res


def kernel(x, W_qkv):
    return _run(x, W_qkv)[0]


# revision 28
# speedup vs baseline: 1.0735x; 1.0031x over previous
"""Trainium2 Bass kernel for nn_Attention_layer_67877663146058.

Computes attn = softmax((x @ Wq.T) @ (x @ Wk.T)^T * hd**-0.5)
for x [8, 1024, 768], W_qkv [2304, 768] -> out [8, 12, 1024, 1024] fp32.
The V third of W_qkv never reaches the output and is not loaded.

Sharding: batch-parallel across the 8 NeuronCores (core b handles batch b,
all 12 heads).

Design notes (evolved from trace analysis of two prior versions):
- v1 was HBM-bound (107% avg HBM util) writing the 50MB fp32 output per
  core. All data is now fp16; the output is written as fp16 scaled by
  1024 (softmax entries down to ~2e-6 would be fp16 subnormals; the
  x1024 shift keeps them normal) and the host upcast multiplies by the
  exact power-of-two 1/1024. HBM traffic: 58MB -> 29MB per core.
- ACT exp is then the pacer. All DVE reduce-variant instructions
  (tensor_scalar+accum, tensor_reduce, tensor_tensor_reduce, bn_stats)
  run at 1x (~1.13us per [128,1024] tile) - only plain copy/scalar ops
  reach the 4x 2-byte mode (~0.49us). GpSimd cannot reduce along the
  free dim at all. So row sums are split:
    * 3 qb-slices per f-tile ("A"): exp in tile-aligned [128,1024] spans
      with the free ACT accumulator (costs +187ns READ_ACCUMULATOR and
      span-overhead fragmentation on ACT),
    * 5 qb-slices per f-tile ("G"): exp in [128,1536] spans (amortizes
      the ~250-cycle ACT per-instruction overhead), sums via the 1x DVE
      identity tensor_scalar(mult,add,accum_out).
- The normalize multiply (y * (1024/sum), fused two-op tensor_scalar,
  ~0.6us on DVE) mostly runs on GpSimd (~1.14us each, but the engine is
  otherwise idle); DVE keeps ~1 per f-tile plus the last f-tile's
  trailing A-slices (short tail). Engines land at roughly ACT 103 /
  DVE 100 / GpSimd 95 us busy - the three-way LP optimum for this
  instruction set.
- Input DMAs are consolidated into 4 fat partition-major transfers:
  each dma_start costs ~0.6us of serial descriptor-gen on the issuing
  sequencer, and 18 separate input DMAs delayed the first projection by
  ~9us in an earlier trace. Projection matmuls for f-tile fi+1 are
  threaded 1-2 instructions at a time between f-tile fi's score spans
  (an interleaved accumulation group only owns its PSUM bank), keeping
  the in-order PE stream from blocking the ACT-paced span pipeline.
- PSUM: 2x [128,512] proj slots + 2x [128,1536] score slots = 8 banks.
- A dependency-free exp at t=0 preloads the ACT spline table (~2.7us).
  The projection matmul stream itself warms the PE HAM clock gate.
"""

import numpy as np
from contextlib import ExitStack

import concourse.bacc as bacc
import concourse.mybir as mybir
import concourse.tile as tile

# bass_utils imports antenv.axon_hooks when BASS_TRACE is set in the
# environment; some images ship an antenv stub without that module. Register
# a no-op fallback so tracing degrades gracefully instead of crashing.
try:
    from antenv.axon_hooks import get_axon_ntff_profile_hook as _g  # noqa: F401
except Exception:
    import sys as _sys
    import types as _types

    _m = _types.ModuleType("antenv.axon_hooks")
    _state = {"h": None}
    _m.set_axon_ntff_profile_hook = lambda h: _state.__setitem__("h", h)
    _m.get_axon_ntff_profile_hook = lambda: _state["h"]
    _sys.modules["antenv.axon_hooks"] = _m
    try:
        import antenv as _antenv

        _antenv.axon_hooks = _m
    except Exception:
        pass

from concourse.bass_utils import run_bass_kernel_spmd

B = 8          # batches == cores
N = 1024       # tokens
E = 768        # embed dim
H = 12         # heads
HD = 64        # head dim
FT = 6         # f-tiles (2 heads per f-tile)
ET = E // 128  # 6 e-tiles
SPAN = 1536    # G-region exp span (one PSUM score slot, 3 banks)
SCALE = HD ** -0.5
OUT_SCALE = 1024.0   # fp16 output holds out*1024 to avoid subnormals

# qb slices per f-tile whose sums come from the ACT accumulator
A_QBS = (0, 3, 6)
# tile indices (2*slot+tt over the fi's emission order) whose normalize
# multiply runs on DVE; the rest run on GpSimd
DVE_MULT = frozenset({5, 11})

_cache = {}


def _build():
    f32 = mybir.dt.float32
    f16 = mybir.dt.float16
    mult = mybir.AluOpType.mult
    add = mybir.AluOpType.add
    Exp = mybir.ActivationFunctionType.Exp
    nc = bacc.Bacc("TRN2", debug=False, num_devices=B)

    # inputs are packed partition-major on the host so each DMA moves
    # multi-KB contiguous runs per partition (128 fat descriptors/DMA)
    xP_d = nc.dram_tensor("xP", [128, ET * N], f16, kind="ExternalInput")
    wA_d = nc.dram_tensor("wA", [128, ET * 256], f16, kind="ExternalInput")
    wB_d = nc.dram_tensor("wB", [128, ET * 1280], f16, kind="ExternalInput")
    out_d = nc.dram_tensor("out", [FT * 8, 128, 2048], f16, kind="ExternalOutput")

    xP_src = xP_d.ap().rearrange("p (t n) -> p t n", t=ET)      # [128,6,1024]
    wA_src = wA_d.ap().rearrange("p (t c) -> p t c", t=ET)      # [128,6,256]
    wB_src = wB_d.ap().rearrange("p (t c) -> p t c", t=ET)      # [128,6,1280]
    out_ap = out_d.ap()

    with ExitStack() as ctx:
        tc = ctx.enter_context(tile.TileContext(nc))
        statics = ctx.enter_context(tc.tile_pool(name="statics", bufs=1))
        ypool = ctx.enter_context(tc.tile_pool(name="ypool", bufs=2))
        spool = ctx.enter_context(tc.tile_pool(name="spool", bufs=2))
        psum = ctx.enter_context(tc.tile_pool(name="psum", bufs=2, space="PSUM"))

        xt = statics.tile([128, ET, N], f16, tag="xt", name="xt")
        # W columns split into two contiguous tiles (f-tile 0 / rest) so
        # each input DMA moves multi-KB runs per partition
        wt0 = statics.tile([128, ET, 256], f16, tag="wt0", name="wt0")
        wtR = statics.tile([128, ET, 1280], f16, tag="wtR", name="wtR")
        # qkt[:, fi, 0, :] = K^T of f-tile fi, qkt[:, fi, 1, :] = Q^T
        qkt = statics.tile([128, FT, 2, N], f16, tag="qkt", name="qkt")

        def w_block(fi, kq, ei):
            if fi == 0:
                return wt0[:, ei, kq * 128:(kq + 1) * 128]
            c = (fi - 1) * 256 + kq * 128
            return wtR[:, ei, c:c + 128]

        # ACT table preload: dependency-free exp at t=0 pulls the ~2.7us
        # ACT_TABLE_LOAD off the critical path of the first real exp.
        warm = spool.tile([128, 1], f32, tag="warm", name="warm")
        nc.vector.memset(warm, 0.0)
        nc.scalar.activation(warm, warm, Exp)

        # Consolidated input loads (4 fat DMAs, 128 descriptors each): x
        # halves and f-tile-0 W columns first so the first projection can
        # start as early as possible.
        nc.sync.dma_start(wt0, wA_src)
        nc.sync.dma_start(xt[:, 0:3, :], xP_src[:, 0:3, :])
        nc.sync.dma_start(xt[:, 3:6, :], xP_src[:, 3:6, :])
        nc.sync.dma_start(wtR, wB_src)

        QUARTERS = [(0, 0), (0, 1), (1, 0), (1, 1)]  # K halves first

        def proj_steps(fi):
            # The projection for f-tile fi as a list of single-instruction
            # closures (6 accumulating matmuls + 1 evacuation cast per
            # quarter). Threading these one or two at a time between score
            # spans keeps the PE from blocking the ACT-paced span stream
            # for ~2us at a stretch (accumulation groups don't need to be
            # contiguous in the PE program; they only own their PSUM bank).
            steps = []
            for qi, (kq, half) in enumerate(QUARTERS):
                # f-tile 0 runs during the fill phase while the score slots
                # are idle: its Q quarters borrow "ps" slots so all four
                # quarters proceed without pp-ring (cast) waits.
                tag = "ps" if (fi == 0 and qi >= 2) else "pp"
                shape = [128, SPAN] if tag == "ps" else [128, 512]
                holder = []
                for ei in range(ET):
                    def mm_step(kq=kq, half=half, ei=ei, holder=holder,
                                tag=tag, shape=shape):
                        if ei == 0:
                            holder.append(psum.tile(
                                shape, f32, tag=tag,
                                name=f"pp{fi}_{kq}_{half}",
                            ))
                        nc.tensor.matmul(
                            holder[0][:, 0:512],
                            lhsT=w_block(fi, kq, ei),
                            rhs=xt[:, ei, half * 512:(half + 1) * 512],
                            start=(ei == 0),
                            stop=(ei == ET - 1),
                            skip_group_check=True,
                        )
                    steps.append(mm_step)

                def cast_step(kq=kq, half=half, holder=holder):
                    nc.vector.tensor_copy(
                        qkt[:, fi, kq, half * 512:(half + 1) * 512],
                        holder[0][:, 0:512],
                    )
                steps.append(cast_step)
            return steps

        def score_mm(ps, off, fi, qb, hh, nh):
            lo = 64 * hh
            nc.tensor.matmul(
                ps[:, off:off + 512],
                lhsT=qkt[lo:lo + 64, fi, 1, qb * 128:(qb + 1) * 128],
                rhs=qkt[lo:lo + 64, fi, 0, nh * 512:(nh + 1) * 512],
                start=True,
                stop=True,
                tile_position=(lo, 0),
            )

        def emit_attn(fi, interleave):
            # y slot s (0..7) holds logical qb slot_qb[s]. A-slices (ACT
            # accumulator, self-contained 1024-col spans) are interleaved
            # BETWEEN span-aligned G-runs (3 slices = 4x1536, 2 slices =
            # 2x1536+1024): the G-region generates ~1.6x more DVE work per
            # ACT span than ACT consumes (identity sums + recip + mult +
            # proj cast), so a contiguous G-region lets the DVE queue grow
            # ~6us deep and the projection cast stuck in it stalls the
            # pp-ring -> PE -> ACT. A-slices underload DVE and drain it.
            # In the last f-tile the A-slices trail so the tail after the
            # final exp is just recip+mult+DMA.
            a_last = fi == FT - 1
            a_qbs = list(A_QBS)
            g_qbs = [q for q in range(8) if q not in A_QBS]
            if a_last:
                segs = [("G", g_qbs[0:3]), ("A", [a_qbs[0]]),
                        ("G", g_qbs[3:5]), ("A", [a_qbs[1]]), ("A", [a_qbs[2]])]
            else:
                segs = [("A", [a_qbs[0]]), ("G", g_qbs[0:3]),
                        ("A", [a_qbs[1]]), ("G", g_qbs[3:5]), ("A", [a_qbs[2]])]
            slot_qb = [qb for _, qbs in segs for qb in qbs]
            y = ypool.tile([128, 16 * N], f16, tag="y", name=f"y{fi}")
            sums = spool.tile([128, 16], f32, tag="sums", name=f"sm{fi}")
            rec = spool.tile([128, 16], f32, tag="rec", name=f"rc{fi}")
            pending = list(interleave)

            def pull(k):
                for fn in pending[:k]:
                    fn()
                del pending[:k]

            def finish_slice(s):
                pull(1)
                qb = slot_qb[s]
                t0 = 2 * s
                nc.vector.reciprocal(rec[:, t0:t0 + 2], sums[:, t0:t0 + 2])
                split_dma = a_last and s == 7
                for tt in (t0, t0 + 1):
                    yt = y[:, tt * N:(tt + 1) * N]
                    dve = (s >= 3) if a_last else (tt in DVE_MULT)
                    eng = nc.vector if dve else nc.gpsimd
                    eng.tensor_scalar(yt, yt, rec[:, tt:tt + 1], OUT_SCALE, mult, mult)
                    if split_dma:
                        nc.sync.dma_start(
                            out_ap[fi * 8 + qb][:, (tt - t0) * N:(tt - t0 + 1) * N],
                            yt,
                        )
                if not split_dma:
                    nc.sync.dma_start(
                        out_ap[fi * 8 + qb], y[:, s * 2048:(s + 1) * 2048]
                    )

            s = 0
            for kind, qbs in segs:
                if kind == "A":
                    qb = qbs[0]
                    for tt in (2 * s, 2 * s + 1):
                        hh = tt % 2
                        ps = psum.tile([128, SPAN], f32, tag="ps", name=f"psA{fi}_{tt}")
                        for nh in range(2):
                            score_mm(ps, nh * 512, fi, qb, hh, nh)
                        nc.scalar.activation(
                            y[:, tt * N:(tt + 1) * N], ps[:, 0:N], Exp, scale=SCALE,
                            accum_out=sums[:, tt:tt + 1],
                        )
                        pull(2)
                    finish_slice(s)
                    s += 1
                else:
                    run0 = s * 2048
                    run1 = run0 + len(qbs) * 2048
                    done = run0
                    for c0 in range(run0, run1, SPAN):
                        L = min(SPAN, run1 - c0)
                        ps = psum.tile([128, SPAN], f32, tag="ps", name=f"psG{fi}_{c0}")
                        for off in range(0, L, 512):
                            g = c0 + off
                            sg = g // 2048
                            score_mm(ps, off, fi, slot_qb[sg],
                                     (g // 1024) % 2, (g // 512) % 2)
                        nc.scalar.activation(
                            y[:, c0:c0 + L], ps[:, 0:L], Exp, scale=SCALE,
                        )
                        pull(2)
                        new_done = ((c0 + L) // N) * N
                        for tt in range(done // N, new_done // N):
                            yt = y[:, tt * N:(tt + 1) * N]
                            nc.vector.tensor_scalar(
                                yt, yt, 1.0, 0.0, mult, add,
                                accum_out=sums[:, tt:tt + 1],
                            )
                            if tt % 2 == 1:
                                finish_slice(tt // 2)
                        done = new_done
                    s += len(qbs)

        # fi0 projection upfront; fi+1's projection threads into fi's attn
        # stream one instruction at a time.
        for fn in proj_steps(0):
            fn()
        for fi in range(FT):
            interleave = proj_steps(fi + 1) if fi + 1 < FT else []
            emit_attn(fi, interleave)

    nc.compile()
    return nc


def _prep_inputs(x, W_qkv):
    x = np.asarray(x, dtype=np.float32)
    W = np.asarray(W_qkv, dtype=np.float32)
    # per-fi W column blocks [K_fi (128) | Q_fi (128)], then packed
    # partition-major: w[p, ei, c] = wT[ei*128+p, c]
    wq = W[0:768].reshape(FT, 128, E)        # Q blocks per f-tile
    wk = W[768:1536].reshape(FT, 128, E)     # K blocks per f-tile
    wkq = np.stack([wk, wq], axis=1)         # [fi, kq, 128, e]
    wT = wkq.transpose(3, 0, 1, 2).reshape(E, 2 * 128 * FT)  # [e, cols]
    wP = wT.reshape(ET, 128, 2 * 128 * FT).transpose(1, 0, 2)  # [p, ei, cols]
    wA = np.ascontiguousarray(wP[:, :, 0:256].reshape(128, -1)).astype(np.float16)
    wB = np.ascontiguousarray(wP[:, :, 256:1536].reshape(128, -1)).astype(np.float16)
    in_maps = []
    for b in range(B):
        xT = x[b].T                           # [e, n]
        xP = np.ascontiguousarray(
            xT.reshape(ET, 128, N).transpose(1, 0, 2).reshape(128, -1)
        ).astype(np.float16)
        in_maps.append({"xP": xP, "wA": wA, "wB": wB})
    return in_maps


def _postprocess(res):
    outs = []
    inv = np.float32(1.0 / OUT_SCALE)
    for r in res.results:
        buf = r["out"]            # [48, 128, 2048] fp16, = out*1024
        buf = np.asarray(buf)
        if buf.dtype != np.float16:
            buf = buf.view(np.float16)
        full = buf.reshape(FT, 8, 128, 2, N).transpose(0, 3, 1, 2, 4)
        full = full.reshape(H, N, N).astype(np.float32) * inv
        outs.append(full)
    return np.stack(outs, axis=0)


def _run(x, W_qkv, trace=False):
    if "nc" not in _cache:
        _cache["nc"] = _build()
    nc = _cache["nc"]
    in_maps = _prep_inputs(x, W_qkv)
    res = run_bass_kernel_spmd(nc, in_maps, core_ids=list(range(B)), trace=trace)
    return _postprocess(res), res


def kernel(x, W_qkv):
    return _run(x, W_qkv)[0]
